# revision 8
# baseline (speedup 1.0000x reference)
"""TRN2 Bass/Tile kernel: 16-head MHA, B=1 S=4096 E=1024, head-sharded over 8 cores.

Sharding: tensor-parallel over heads. Core c owns heads {2c, 2c+1}: columns
[128c, 128(c+1)) of Wq/Wk/Wv (+bias slices) and rows [128c, 128(c+1)) of Wo.
Each core computes attention for its 2 heads and a partial out-projection
[S, E] in fp16; the host sums the 8 partials and adds bo.

v2 pipeline (all fp16 matmuls, fp32 PSUM):
  A) QT/KT [128ch, S] = W^T x (row 0:64 = head0 ch, 64:128 = head1 ch), VT same.
     Bias folded into the PSUM->SBUF evacuation (tensor_scalar_add).
  B) V65[h] [128k, kt, 65] = [V_h | ones] via DMA transpose of VT.
  C) scores^T [k, q]: TWO row-tiled concurrent matmuls (c=64 per head,
     tile_position (0,0)/(64,0)) into one [128, 1024] PSUM pair.
     exp: even kt on ACT (exact, scale=1/8), odd kt on DVE via fp16
     Schraudolph (one tensor_scalar mult+add -> int16 bitcast).
     PV: [65, 512] += V65^T ex accumulated over kt; row 64 = softmax denom.
  D) recip(l) -> DRAM bounce partition-broadcast -> scale -> ATT [128ch, S]
     -> out-proj ATT^T Wo per 128-q tile -> fp16 out.
"""

import sys

for _p in ("/opt/trn_rl_repo", "/opt/pypackages"):
    if _p not in sys.path:
        sys.path.append(_p)

import numpy as np

EMBED = 1024
N_CORES = 8
HC = EMBED // N_CORES  # 128 channels = 2 heads per core
DH = 64                # head dim
SEQ = 4096

# fp16 Schraudolph: exp(s/8) ~= bitcast_f16(int16(A16*s + B16))
A16 = 1024 * 1.4426950408889634 / 8.0
B16 = 15360.0 - 44.25

_NC_CACHE = {}


def _build_nc(S=SEQ, E=EMBED):
    from contextlib import ExitStack

    import concourse.bass as bass
    import concourse.mybir as mybir
    import concourse.tile as tile
    from concourse import bacc

    F32 = mybir.dt.float32
    F16 = mybir.dt.float16
    I16 = mybir.dt.int16

    ET = E // 128      # 8 contraction tiles for projections
    NSC = S // 512     # 8 S-chunks
    NKT = S // 128     # 32 key tiles
    NQS = 512 // 128   # 128-q subtiles per chunk

    nc = bacc.Bacc()
    xT = nc.declare_dram_parameter("xT", [E, S], F16, isOutput=False)
    wq = nc.declare_dram_parameter("wq", [E, HC], F16, isOutput=False)
    wk = nc.declare_dram_parameter("wk", [E, HC], F16, isOutput=False)
    wv = nc.declare_dram_parameter("wv", [E, HC], F16, isOutput=False)
    bq = nc.declare_dram_parameter("bq", [HC, 1], F32, isOutput=False)
    bk = nc.declare_dram_parameter("bk", [HC, 1], F32, isOutput=False)
    bv = nc.declare_dram_parameter("bv", [HC, 1], F32, isOutput=False)
    wo = nc.declare_dram_parameter("wo", [HC, E], F16, isOutput=False)
    out = nc.declare_dram_parameter("out", [S, E], F16, isOutput=True)

    with tile.TileContext(nc) as tc, ExitStack() as ctx:
        wpool = ctx.enter_context(tc.tile_pool(name="w", bufs=1))
        xpool = ctx.enter_context(tc.tile_pool(name="x", bufs=1))
        bigpool = ctx.enter_context(tc.tile_pool(name="big", bufs=1))
        expool = ctx.enter_context(tc.tile_pool(name="e", bufs=4))
        rpool = ctx.enter_context(tc.tile_pool(name="r", bufs=2))
        opool = ctx.enter_context(tc.tile_pool(name="o", bufs=3))
        dpool = ctx.enter_context(tc.tile_pool(name="d", bufs=2, space="DRAM"))
        # PSUM: spsum 2x[128,1024]=4 banks; pv 4x[128,512]=4 banks
        spsum = ctx.enter_context(tc.tile_pool(name="sp", bufs=2, space="PSUM"))
        pvpsum = ctx.enter_context(tc.tile_pool(name="pv", bufs=4, space="PSUM"))

        # --- weights / biases resident ---
        w_sb = {}
        for name, src in (("wq", wq), ("wk", wk), ("wv", wv)):
            t = wpool.tile([128, ET, HC], F16, tag=name, name=name)
            nc.sync.dma_start(out=t, in_=src.rearrange("(a p) c -> p a c", p=128))
            w_sb[name] = t
        wo_sb = wpool.tile([HC, E], F16, tag="wo")
        nc.sync.dma_start(out=wo_sb, in_=wo[:, :])
        b_sb = {}
        for name, src in (("bq", bq), ("bk", bk), ("bv", bv)):
            t = wpool.tile([HC, 1], F32, tag=name, name=name)
            nc.sync.dma_start(out=t, in_=src[:, :])
            b_sb[name] = t

        # --- resident activations ---
        xt = []
        for et in range(ET):
            t = xpool.tile([128, S], F16, tag=f"xt{et}", name=f"xt{et}")
            nc.sync.dma_start(out=t, in_=xT[et * 128:(et + 1) * 128, :])
            xt.append(t)
        QKT = bigpool.tile([128, 2 * S], F16, tag="qkt")   # [:, 0:S]=Q, [S:2S]=K
        VT = bigpool.tile([128, S], F16, tag="vt")
        ATT = bigpool.tile([128, S], F16, tag="att")
        # padded to 128 so DMA-transpose outputs land 128-aligned
        V65 = [bigpool.tile([128, NKT, 128], F16, tag=f"v65_{h}", name=f"v65_{h}")
               for h in range(2)]
        for h in range(2):
            nc.vector.memset(V65[h][:, :, 64:65], 1.0)

        # --- stage A: projections, 512-wide S chunks ---
        for sc in range(NSC):
            ssl = slice(sc * 512, (sc + 1) * 512)
            big1 = spsum.tile([128, 1024], F32, tag="sb")
            big2 = pvpsum.tile([128, 512], F32, tag="pv")
            for et in range(ET):
                xsl = xt[et][:, ssl]
                first, last = et == 0, et == ET - 1
                nc.tensor.matmul(big1[:, 0:512], lhsT=w_sb["wq"][:, et, :],
                                 rhs=xsl, start=first, stop=last)
                nc.tensor.matmul(big1[:, 512:1024], lhsT=w_sb["wk"][:, et, :],
                                 rhs=xsl, start=first, stop=last)
                nc.tensor.matmul(big2, lhsT=w_sb["wv"][:, et, :],
                                 rhs=xsl, start=first, stop=last)
            nc.vector.tensor_scalar_add(QKT[:, ssl], big1[:, 0:512], b_sb["bq"])
            nc.vector.tensor_scalar_add(QKT[:, S + sc * 512:S + (sc + 1) * 512],
                                        big1[:, 512:1024], b_sb["bk"])
            nc.vector.tensor_scalar_add(VT[:, ssl], big2, b_sb["bv"])
            # stage B: V65 via DMA transpose of freshly-written VT columns
            for h in range(2):
                nc.sync.dma_start_transpose(
                    out=V65[h][:, 4 * sc:4 * sc + 4, 0:64],
                    in_=VT[h * 64:(h + 1) * 64, ssl],
                )

        # --- stage C+D: attention per 512-q chunk; out-proj deferred 1 chunk ---
        def outproj(qq):
            for qs in range(NQS):
                po = spsum.tile([128, 1024], F32, tag="sb", name="po")
                at = ATT[:, qq * 512 + qs * 128:qq * 512 + (qs + 1) * 128]
                nc.tensor.matmul(po[:, 0:512], lhsT=at, rhs=wo_sb[:, 0:512],
                                 start=True, stop=True)
                nc.tensor.matmul(po[:, 512:1024], lhsT=at, rhs=wo_sb[:, 512:1024],
                                 start=True, stop=True)
                osb = opool.tile([128, 1024], F16, tag="osb", name="osb")
                nc.scalar.copy(osb, po)
                nc.sync.dma_start(
                    out=out[qq * 512 + qs * 128:qq * 512 + (qs + 1) * 128, :],
                    in_=osb,
                )

        for qc in range(NSC):
            qsl = slice(qc * 512, (qc + 1) * 512)
            pv = [pvpsum.tile([128, 512], F32, tag="pv", name=f"pv{h}")
                  for h in range(2)]
            for kt in range(NKT):
                sb = spsum.tile([128, 1024], F32, tag="sb")
                for h in range(2):
                    hsl = slice(h * 64, (h + 1) * 64)
                    nc.tensor.matmul(
                        sb[:, h * 512:(h + 1) * 512],
                        lhsT=QKT[hsl, S + kt * 128:S + (kt + 1) * 128],
                        rhs=QKT[hsl, qsl],
                        start=True, stop=True,
                    )
                ex = expool.tile([128, 1024], F16, tag="ex")
                if kt % 2 == 0:
                    nc.scalar.activation(
                        ex, sb, mybir.ActivationFunctionType.Exp, scale=0.125)
                else:
                    nc.vector.tensor_scalar(
                        ex[:, :].bitcast(I16), sb[:, :], A16, B16,
                        mybir.AluOpType.mult, mybir.AluOpType.add)
                for h in range(2):
                    nc.tensor.matmul(
                        pv[h][0:65, :], lhsT=V65[h][:, kt, 0:65],
                        rhs=ex[:, h * 512:(h + 1) * 512],
                        start=(kt == 0), stop=(kt == NKT - 1),
                    )
            # normalize: denom rows -> DRAM -> [128, 8] respread -> recip ->
            # DRAM -> partition-broadcast. Avoids 1-lane DVE reciprocal.
            lr = rpool.tile([33, 512], F32, tag="lr", name="lr")
            for h in range(2):
                nc.scalar.copy(lr[32 * h:32 * h + 1, :], pv[h][64:65, :])
            scr = dpool.tile([2, 512], F32, tag="scr")
            nc.sync.dma_start(
                out=scr,
                in_=bass.AP(tensor=lr[:, :].tensor, offset=lr[:, :].offset,
                            ap=[[512 * 32, 2], [1, 512]]),
            )
            rsp = rpool.tile([128, 8], F32, tag="rsp", name="rsp")
            nc.sync.dma_start(
                out=rsp,
                in_=bass.AP(tensor=scr.tensor, offset=scr.offset,
                            ap=[[1, 128], [128, 8]]),
            )
            rsp2 = rpool.tile([128, 8], F32, tag="rsp2", name="rsp2")
            nc.vector.reciprocal(rsp2, rsp)
            scr2 = dpool.tile([2, 512], F32, tag="scr2")
            nc.sync.dma_start(
                out=bass.AP(tensor=scr2.tensor, offset=scr2.offset,
                            ap=[[1, 128], [128, 8]]),
                in_=rsp2,
            )
            bc = rpool.tile([128, 512], F32, tag="bc", name="bc")
            for h in range(2):
                nc.sync.dma_start(
                    out=bc[h * 64:(h + 1) * 64, :],
                    in_=bass.AP(tensor=scr2.tensor, offset=scr2.offset + h * 512,
                                ap=[[0, 64], [1, 512]]),
                )
            for h in range(2):
                nc.vector.tensor_mul(ATT[h * 64:(h + 1) * 64, qsl],
                                     pv[h][0:64, :], bc[h * 64:(h + 1) * 64, :])
            if qc > 0:
                outproj(qc - 1)
        outproj(NSC - 1)
    nc.finalize()
    return nc


def _get_nc(S=SEQ):
    key = S
    if key not in _NC_CACHE:
        _NC_CACHE[key] = _build_nc(S=S)
    return _NC_CACHE[key]


def _make_in_maps(x, Wq, bq, Wk, bk, Wv, bv, Wo):
    xT = np.ascontiguousarray(np.asarray(x, np.float32)[0].T.astype(np.float16))
    Wq, Wk, Wv, Wo = (np.asarray(a, np.float32).astype(np.float16)
                      for a in (Wq, Wk, Wv, Wo))
    bq, bk, bv = (np.asarray(a, np.float32) for a in (bq, bk, bv))
    in_maps = []
    for c in range(N_CORES):
        sl = slice(c * HC, (c + 1) * HC)
        in_maps.append({
            "xT": xT,
            "wq": np.ascontiguousarray(Wq[:, sl]),
            "wk": np.ascontiguousarray(Wk[:, sl]),
            "wv": np.ascontiguousarray(Wv[:, sl]),
            "bq": np.ascontiguousarray(bq[sl]).reshape(HC, 1),
            "bk": np.ascontiguousarray(bk[sl]).reshape(HC, 1),
            "bv": np.ascontiguousarray(bv[sl]).reshape(HC, 1),
            "wo": np.ascontiguousarray(Wo[sl, :]),
        })
    return in_maps


def run(inputs, trace=False):
    """Run the kernel; returns (out [1,S,E] float32, BassKernelResults)."""
    from concourse.bass_utils import run_bass_kernel_spmd

    nc = _get_nc()
    in_maps = _make_in_maps(
        inputs["x"], inputs["Wq"], inputs["bq"], inputs["Wk"], inputs["bk"],
        inputs["Wv"], inputs["bv"], inputs["Wo"],
    )
    res = run_bass_kernel_spmd(
        nc, in_maps, core_ids=list(range(N_CORES)), trace=trace
    )
    acc = np.zeros((SEQ, EMBED), np.float64)
    for c in range(N_CORES):
        acc += res.results[c]["out"].astype(np.float64)
    acc += np.asarray(inputs["bo"], np.float64)
    return acc.astype(np.float32).reshape(1, SEQ, EMBED), res


def kernel(x, Wq, bq, Wk, bk, Wv, bv, Wo, bo):
    out, _ = run(dict(x=x, Wq=Wq, bq=bq, Wk=Wk, bk=bk, Wv=Wv, bv=bv, Wo=Wo, bo=bo))
    return out


# revision 12
# speedup vs baseline: 1.0985x; 1.0985x over previous
"""TRN2 Bass/Tile kernel: 16-head MHA, B=1 S=4096 E=1024, head-sharded over 8 cores.

Sharding: tensor-parallel over heads. Core c owns heads {2c, 2c+1}: columns
[128c, 128(c+1)) of Wq/Wk/Wv (+bias slices) and rows [128c, 128(c+1)) of Wo.
Each core computes attention for its 2 heads and a partial out-projection
[S, E] in fp16; the host sums the 8 partials and adds bo.

v2 pipeline (all fp16 matmuls, fp32 PSUM):
  A) QT/KT [128ch, S] = W^T x (row 0:64 = head0 ch, 64:128 = head1 ch), VT same.
     Bias folded into the PSUM->SBUF evacuation (tensor_scalar_add).
  B) V65[h] [128k, kt, 65] = [V_h | ones] via DMA transpose of VT.
  C) scores^T [k, q]: TWO row-tiled concurrent matmuls (c=64 per head,
     tile_position (0,0)/(64,0)) into one [128, 1024] PSUM pair.
     exp: even kt on ACT (exact, scale=1/8), odd kt on DVE via fp16
     Schraudolph (one tensor_scalar mult+add -> int16 bitcast).
     PV: [65, 512] += V65^T ex accumulated over kt; row 64 = softmax denom.
  D) recip(l) -> DRAM bounce partition-broadcast -> scale -> ATT [128ch, S]
     -> out-proj ATT^T Wo per 128-q tile -> fp16 out.
"""

import sys

for _p in ("/opt/trn_rl_repo", "/opt/pypackages"):
    if _p not in sys.path:
        sys.path.append(_p)

import numpy as np

EMBED = 1024
N_CORES = 8
HC = EMBED // N_CORES  # 128 channels = 2 heads per core
DH = 64                # head dim
SEQ = 4096

# fp16 Schraudolph: exp(s/8) ~= bitcast_f16(int16(A16*s + B16))
A16 = 1024 * 1.4426950408889634 / 8.0
B16 = 15360.0 - 44.25

_NC_CACHE = {}


def _build_nc(S=SEQ, E=EMBED):
    from contextlib import ExitStack

    import concourse.bass as bass
    import concourse.mybir as mybir
    import concourse.tile as tile
    from concourse import bacc

    F32 = mybir.dt.float32
    F16 = mybir.dt.float16
    I16 = mybir.dt.int16

    ET = E // 128      # 8 contraction tiles for projections
    NSC = S // 512     # 8 S-chunks
    NKT = S // 128     # 32 key tiles
    NQS = 512 // 128   # 128-q subtiles per chunk

    nc = bacc.Bacc()
    xT = nc.declare_dram_parameter("xT", [E, S], F16, isOutput=False)
    wq = nc.declare_dram_parameter("wq", [E, HC], F16, isOutput=False)
    wk = nc.declare_dram_parameter("wk", [E, HC], F16, isOutput=False)
    wv = nc.declare_dram_parameter("wv", [E, HC], F16, isOutput=False)
    bq = nc.declare_dram_parameter("bq", [HC, 1], F32, isOutput=False)
    bk = nc.declare_dram_parameter("bk", [HC, 1], F32, isOutput=False)
    bv = nc.declare_dram_parameter("bv", [HC, 1], F32, isOutput=False)
    wo = nc.declare_dram_parameter("wo", [HC, E], F16, isOutput=False)
    out = nc.declare_dram_parameter("out", [S, E], F16, isOutput=True)

    with tile.TileContext(nc) as tc, ExitStack() as ctx:
        wpool = ctx.enter_context(tc.tile_pool(name="w", bufs=1))
        xpool = ctx.enter_context(tc.tile_pool(name="x", bufs=1))
        bigpool = ctx.enter_context(tc.tile_pool(name="big", bufs=1))
        expool = ctx.enter_context(tc.tile_pool(name="e", bufs=4))
        rpool = ctx.enter_context(tc.tile_pool(name="r", bufs=2))
        opool = ctx.enter_context(tc.tile_pool(name="o", bufs=3))
        dpool = ctx.enter_context(tc.tile_pool(name="d", bufs=2, space="DRAM"))
        # PSUM: spsum 2x[128,1024]=4 banks; pv 4x[128,512]=4 banks
        spsum = ctx.enter_context(tc.tile_pool(name="sp", bufs=2, space="PSUM"))
        pvpsum = ctx.enter_context(tc.tile_pool(name="pv", bufs=4, space="PSUM"))

        # --- weights / biases resident ---
        w_sb = {}
        for name, src in (("wq", wq), ("wk", wk), ("wv", wv)):
            t = wpool.tile([128, ET, HC], F16, tag=name, name=name)
            nc.sync.dma_start(out=t, in_=src.rearrange("(a p) c -> p a c", p=128))
            w_sb[name] = t
        wo_sb = wpool.tile([HC, E], F16, tag="wo")
        nc.sync.dma_start(out=wo_sb, in_=wo[:, :])
        b_sb = {}
        for name, src in (("bq", bq), ("bk", bk), ("bv", bv)):
            t = wpool.tile([HC, 1], F32, tag=name, name=name)
            nc.sync.dma_start(out=t, in_=src[:, :])
            b_sb[name] = t

        # --- resident activations ---
        xt = []
        for et in range(ET):
            t = xpool.tile([128, S], F16, tag=f"xt{et}", name=f"xt{et}")
            nc.sync.dma_start(out=t, in_=xT[et * 128:(et + 1) * 128, :])
            xt.append(t)
        QKT = bigpool.tile([128, 2 * S], F16, tag="qkt")   # [:, 0:S]=Q, [S:2S]=K
        VT = bigpool.tile([128, S], F16, tag="vt")
        ATT = bigpool.tile([128, S], F16, tag="att")
        # padded to 128 so DMA-transpose outputs land 128-aligned
        V65 = [bigpool.tile([128, NKT, 128], F16, tag=f"v65_{h}", name=f"v65_{h}")
               for h in range(2)]
        for h in range(2):
            nc.vector.memset(V65[h][:, :, 64:65], 1.0)

        # --- stage A: projections, 512-wide S chunks ---
        for sc in range(NSC):
            ssl = slice(sc * 512, (sc + 1) * 512)
            big1 = spsum.tile([128, 1024], F32, tag="sb")
            big2 = pvpsum.tile([128, 512], F32, tag="pv")
            for et in range(ET):
                xsl = xt[et][:, ssl]
                first, last = et == 0, et == ET - 1
                nc.tensor.matmul(big1[:, 0:512], lhsT=w_sb["wq"][:, et, :],
                                 rhs=xsl, start=first, stop=last)
                nc.tensor.matmul(big1[:, 512:1024], lhsT=w_sb["wk"][:, et, :],
                                 rhs=xsl, start=first, stop=last)
                nc.tensor.matmul(big2, lhsT=w_sb["wv"][:, et, :],
                                 rhs=xsl, start=first, stop=last)
            nc.vector.tensor_scalar_add(QKT[:, ssl], big1[:, 0:512], b_sb["bq"])
            nc.vector.tensor_scalar_add(QKT[:, S + sc * 512:S + (sc + 1) * 512],
                                        big1[:, 512:1024], b_sb["bk"])
            nc.vector.tensor_scalar_add(VT[:, ssl], big2, b_sb["bv"])
            # stage B: V65 via DMA transpose of freshly-written VT columns
            for h in range(2):
                nc.sync.dma_start_transpose(
                    out=V65[h][:, 4 * sc:4 * sc + 4, 0:64],
                    in_=VT[h * 64:(h + 1) * 64, ssl],
                )

        # --- stage C+D: attention per 512-q chunk; out-proj deferred 1 chunk ---
        def outproj(qq):
            for qs in range(NQS):
                po = spsum.tile([128, 1024], F32, tag="sb", name="po")
                at = ATT[:, qq * 512 + qs * 128:qq * 512 + (qs + 1) * 128]
                nc.tensor.matmul(po[:, 0:512], lhsT=at, rhs=wo_sb[:, 0:512],
                                 start=True, stop=True)
                nc.tensor.matmul(po[:, 512:1024], lhsT=at, rhs=wo_sb[:, 512:1024],
                                 start=True, stop=True)
                osb = opool.tile([128, 1024], F16, tag="osb", name="osb")
                nc.scalar.copy(osb, po)
                nc.sync.dma_start(
                    out=out[qq * 512 + qs * 128:qq * 512 + (qs + 1) * 128, :],
                    in_=osb,
                )

        for qc in range(NSC):
            qsl = slice(qc * 512, (qc + 1) * 512)
            pv = [pvpsum.tile([128, 512], F32, tag="pv", name=f"pv{h}")
                  for h in range(2)]
            exq = {}

            def pv_mm(kt):
                ex = exq.pop(kt)
                for h in range(2):
                    nc.tensor.matmul(
                        pv[h][0:65, :], lhsT=V65[h][:, kt, 0:65],
                        rhs=ex[:, h * 512:(h + 1) * 512],
                        start=(kt == 0), stop=(kt == NKT - 1),
                    )

            for kt in range(NKT):
                sb = spsum.tile([128, 1024], F32, tag="sb")
                for h in range(2):
                    hsl = slice(h * 64, (h + 1) * 64)
                    nc.tensor.matmul(
                        sb[:, h * 512:(h + 1) * 512],
                        lhsT=QKT[hsl, S + kt * 128:S + (kt + 1) * 128],
                        rhs=QKT[hsl, qsl],
                        start=True, stop=True,
                    )
                ex = expool.tile([128, 1024], F16, tag="ex")
                if kt % 2 == 0:
                    nc.scalar.activation(
                        ex, sb, mybir.ActivationFunctionType.Exp, scale=0.125)
                else:
                    nc.vector.tensor_scalar(
                        ex[:, :].bitcast(I16), sb[:, :], A16, B16,
                        mybir.AluOpType.mult, mybir.AluOpType.add)
                exq[kt] = ex
                # PV runs 2 slots behind so exp latency stays hidden
                if kt >= 2:
                    pv_mm(kt - 2)
                if kt == 6 and qc > 0:
                    outproj(qc - 1)
            pv_mm(NKT - 2)
            pv_mm(NKT - 1)
            # normalize: denom rows -> DRAM -> [128, 8] respread -> recip ->
            # DRAM -> partition-broadcast. Avoids 1-lane DVE reciprocal.
            lr = rpool.tile([33, 512], F32, tag="lr", name="lr")
            for h in range(2):
                nc.vector.tensor_copy(lr[32 * h:32 * h + 1, :], pv[h][64:65, :])
            scr = dpool.tile([2, 512], F32, tag="scr")
            nc.sync.dma_start(
                out=scr,
                in_=bass.AP(tensor=lr[:, :].tensor, offset=lr[:, :].offset,
                            ap=[[512 * 32, 2], [1, 512]]),
            )
            rsp = rpool.tile([128, 8], F32, tag="rsp", name="rsp")
            nc.sync.dma_start(
                out=rsp,
                in_=bass.AP(tensor=scr.tensor, offset=scr.offset,
                            ap=[[1, 128], [128, 8]]),
            )
            rsp2 = rpool.tile([128, 8], F32, tag="rsp2", name="rsp2")
            nc.vector.reciprocal(rsp2, rsp)
            scr2 = dpool.tile([2, 512], F32, tag="scr2")
            nc.sync.dma_start(
                out=bass.AP(tensor=scr2.tensor, offset=scr2.offset,
                            ap=[[1, 128], [128, 8]]),
                in_=rsp2,
            )
            bc = rpool.tile([128, 512], F32, tag="bc", name="bc")
            for h in range(2):
                nc.sync.dma_start(
                    out=bc[h * 64:(h + 1) * 64, :],
                    in_=bass.AP(tensor=scr2.tensor, offset=scr2.offset + h * 512,
                                ap=[[0, 64], [1, 512]]),
                )
            for h in range(2):
                nc.vector.tensor_mul(ATT[h * 64:(h + 1) * 64, qsl],
                                     pv[h][0:64, :], bc[h * 64:(h + 1) * 64, :])
        outproj(NSC - 1)
    nc.finalize()
    return nc


def _get_nc(S=SEQ):
    key = S
    if key not in _NC_CACHE:
        _NC_CACHE[key] = _build_nc(S=S)
    return _NC_CACHE[key]


def _make_in_maps(x, Wq, bq, Wk, bk, Wv, bv, Wo):
    xT = np.ascontiguousarray(np.asarray(x, np.float32)[0].T.astype(np.float16))
    Wq, Wk, Wv, Wo = (np.asarray(a, np.float32).astype(np.float16)
                      for a in (Wq, Wk, Wv, Wo))
    bq, bk, bv = (np.asarray(a, np.float32) for a in (bq, bk, bv))
    in_maps = []
    for c in range(N_CORES):
        sl = slice(c * HC, (c + 1) * HC)
        in_maps.append({
            "xT": xT,
            "wq": np.ascontiguousarray(Wq[:, sl]),
            "wk": np.ascontiguousarray(Wk[:, sl]),
            "wv": np.ascontiguousarray(Wv[:, sl]),
            "bq": np.ascontiguousarray(bq[sl]).reshape(HC, 1),
            "bk": np.ascontiguousarray(bk[sl]).reshape(HC, 1),
            "bv": np.ascontiguousarray(bv[sl]).reshape(HC, 1),
            "wo": np.ascontiguousarray(Wo[sl, :]),
        })
    return in_maps


def run(inputs, trace=False):
    """Run the kernel; returns (out [1,S,E] float32, BassKernelResults)."""
    from concourse.bass_utils import run_bass_kernel_spmd

    nc = _get_nc()
    in_maps = _make_in_maps(
        inputs["x"], inputs["Wq"], inputs["bq"], inputs["Wk"], inputs["bk"],
        inputs["Wv"], inputs["bv"], inputs["Wo"],
    )
    res = run_bass_kernel_spmd(
        nc, in_maps, core_ids=list(range(N_CORES)), trace=trace
    )
    acc = np.zeros((SEQ, EMBED), np.float64)
    for c in range(N_CORES):
        acc += res.results[c]["out"].astype(np.float64)
    acc += np.asarray(inputs["bo"], np.float64)
    return acc.astype(np.float32).reshape(1, SEQ, EMBED), res


def kernel(x, Wq, bq, Wk, bk, Wv, bv, Wo, bo):
    out, _ = run(dict(x=x, Wq=Wq, bq=bq, Wk=Wk, bk=bk, Wv=Wv, bv=bv, Wo=Wo, bo=bo))
    return out


# revision 16
# speedup vs baseline: 1.1472x; 1.0443x over previous
"""TRN2 Bass/Tile kernel: 16-head MHA, B=1 S=4096 E=1024, head-sharded over 8 cores.

Sharding: tensor-parallel over heads. Core c owns heads {2c, 2c+1}: columns
[128c, 128(c+1)) of Wq/Wk/Wv (+bias slices) and rows [128c, 128(c+1)) of Wo.
Each core computes attention for its 2 heads and a partial out-projection
[S, E] in fp16; the host sums the 8 partials and adds bo.

v2 pipeline (all fp16 matmuls, fp32 PSUM):
  A) QT/KT [128ch, S] = W^T x (row 0:64 = head0 ch, 64:128 = head1 ch), VT same.
     Bias folded into the PSUM->SBUF evacuation (tensor_scalar_add).
  B) V65[h] [128k, kt, 65] = [V_h | ones] via DMA transpose of VT.
  C) scores^T [k, q]: TWO row-tiled concurrent matmuls (c=64 per head,
     tile_position (0,0)/(64,0)) into one [128, 1024] PSUM pair.
     exp: even kt on ACT (exact, scale=1/8), odd kt on DVE via fp16
     Schraudolph (one tensor_scalar mult+add -> int16 bitcast).
     PV: [65, 512] += V65^T ex accumulated over kt; row 64 = softmax denom.
  D) recip(l) -> DRAM bounce partition-broadcast -> scale -> ATT [128ch, S]
     -> out-proj ATT^T Wo per 128-q tile -> fp16 out.
"""

import sys

for _p in ("/opt/trn_rl_repo", "/opt/pypackages"):
    if _p not in sys.path:
        sys.path.append(_p)

import numpy as np

EMBED = 1024
N_CORES = 8
HC = EMBED // N_CORES  # 128 channels = 2 heads per core
DH = 64                # head dim
SEQ = 4096

# fp16 Schraudolph: exp(s/8) ~= bitcast_f16(int16(A16*s + B16))
A16 = 1024 * 1.4426950408889634 / 8.0
B16 = 15360.0 - 44.25

_NC_CACHE = {}


def _build_nc(S=SEQ, E=EMBED):
    from contextlib import ExitStack

    import concourse.bass as bass
    import concourse.mybir as mybir
    import concourse.tile as tile
    from concourse import bacc

    F32 = mybir.dt.float32
    F16 = mybir.dt.float16
    I16 = mybir.dt.int16

    ET = E // 128      # 8 contraction tiles for projections
    NSC = S // 512     # 8 S-chunks
    NKT = S // 128     # 32 key tiles
    NQS = 512 // 128   # 128-q subtiles per chunk

    nc = bacc.Bacc()
    xT = nc.declare_dram_parameter("xT", [E, S], F16, isOutput=False)
    wq = nc.declare_dram_parameter("wq", [E, HC], F16, isOutput=False)
    wk = nc.declare_dram_parameter("wk", [E, HC], F16, isOutput=False)
    wv = nc.declare_dram_parameter("wv", [E, HC], F16, isOutput=False)
    bq = nc.declare_dram_parameter("bq", [HC, 1], F32, isOutput=False)
    bk = nc.declare_dram_parameter("bk", [HC, 1], F32, isOutput=False)
    bv = nc.declare_dram_parameter("bv", [HC, 1], F32, isOutput=False)
    wo = nc.declare_dram_parameter("wo", [HC, E], F16, isOutput=False)
    out = nc.declare_dram_parameter("out", [S, E], F16, isOutput=True)

    with tile.TileContext(nc) as tc, ExitStack() as ctx:
        wpool = ctx.enter_context(tc.tile_pool(name="w", bufs=1))
        xpool = ctx.enter_context(tc.tile_pool(name="x", bufs=1))
        bigpool = ctx.enter_context(tc.tile_pool(name="big", bufs=1))
        expool = ctx.enter_context(tc.tile_pool(name="e", bufs=4))
        rpool = ctx.enter_context(tc.tile_pool(name="r", bufs=2))
        opool = ctx.enter_context(tc.tile_pool(name="o", bufs=3))
        dpool = ctx.enter_context(tc.tile_pool(name="d", bufs=2, space="DRAM"))
        # PSUM: spsum 2x[128,1024]=4 banks; pv 4x[128,512]=4 banks
        spsum = ctx.enter_context(tc.tile_pool(name="sp", bufs=2, space="PSUM"))
        pvpsum = ctx.enter_context(tc.tile_pool(name="pv", bufs=4, space="PSUM"))

        # --- weights / biases resident ---
        w_sb = {}
        for name, src in (("wq", wq), ("wk", wk), ("wv", wv)):
            t = wpool.tile([128, ET, HC], F16, tag=name, name=name)
            nc.sync.dma_start(out=t, in_=src.rearrange("(a p) c -> p a c", p=128))
            w_sb[name] = t
        wo_sb = wpool.tile([HC, E], F16, tag="wo")
        nc.sync.dma_start(out=wo_sb, in_=wo[:, :])
        b_sb = {}
        for name, src in (("bq", bq), ("bk", bk), ("bv", bv)):
            t = wpool.tile([HC, 1], F32, tag=name, name=name)
            nc.sync.dma_start(out=t, in_=src[:, :])
            b_sb[name] = t

        # --- resident activations ---
        xt = []
        for et in range(ET):
            t = xpool.tile([128, S], F16, tag=f"xt{et}", name=f"xt{et}")
            nc.sync.dma_start(out=t, in_=xT[et * 128:(et + 1) * 128, :])
            xt.append(t)
        QKT = bigpool.tile([128, 2 * S], F16, tag="qkt")   # [:, 0:S]=Q, [S:2S]=K
        VT = bigpool.tile([128, S], F16, tag="vt")
        ATT = bigpool.tile([128, S], F16, tag="att")
        # padded to 128 so DMA-transpose outputs land 128-aligned
        V65 = [bigpool.tile([128, NKT, 128], F16, tag=f"v65_{h}", name=f"v65_{h}")
               for h in range(2)]
        for h in range(2):
            nc.vector.memset(V65[h][:, :, 64:65], 1.0)

        # --- stage A: projections, 512-wide S chunks ---
        for sc in range(NSC):
            ssl = slice(sc * 512, (sc + 1) * 512)
            big1 = spsum.tile([128, 1024], F32, tag="sb")
            big2 = pvpsum.tile([128, 512], F32, tag="pv")
            for et in range(ET):
                xsl = xt[et][:, ssl]
                first, last = et == 0, et == ET - 1
                nc.tensor.matmul(big1[:, 0:512], lhsT=w_sb["wq"][:, et, :],
                                 rhs=xsl, start=first, stop=last)
                nc.tensor.matmul(big1[:, 512:1024], lhsT=w_sb["wk"][:, et, :],
                                 rhs=xsl, start=first, stop=last)
                nc.tensor.matmul(big2, lhsT=w_sb["wv"][:, et, :],
                                 rhs=xsl, start=first, stop=last)
            nc.vector.tensor_scalar_add(QKT[:, ssl], big1[:, 0:512], b_sb["bq"])
            nc.vector.tensor_scalar_add(QKT[:, S + sc * 512:S + (sc + 1) * 512],
                                        big1[:, 512:1024], b_sb["bk"])
            nc.vector.tensor_scalar_add(VT[:, ssl], big2, b_sb["bv"])
            # stage B: V65 via DMA transpose of freshly-written VT columns
            for h in range(2):
                nc.sync.dma_start_transpose(
                    out=V65[h][:, 4 * sc:4 * sc + 4, 0:64],
                    in_=VT[h * 64:(h + 1) * 64, ssl],
                )

        # --- stage C+D: attention per 512-q chunk; out-proj deferred 1 chunk ---
        def outproj_qs(qq, qs):
            po = spsum.tile([128, 1024], F32, tag="sb", name="po")
            at = ATT[:, qq * 512 + qs * 128:qq * 512 + (qs + 1) * 128]
            nc.tensor.matmul(po[:, 0:512], lhsT=at, rhs=wo_sb[:, 0:512],
                             start=True, stop=True)
            nc.tensor.matmul(po[:, 512:1024], lhsT=at, rhs=wo_sb[:, 512:1024],
                             start=True, stop=True)
            osb = opool.tile([128, 1024], F16, tag="osb", name="osb")
            # alternate the PSUM evacuation engine so neither exp stream stalls
            if qs % 2 == 0:
                nc.scalar.copy(osb, po)
            else:
                nc.vector.tensor_copy(osb, po)
            nc.sync.dma_start(
                out=out[qq * 512 + qs * 128:qq * 512 + (qs + 1) * 128, :],
                in_=osb,
            )

        for qc in range(NSC):
            qsl = slice(qc * 512, (qc + 1) * 512)
            pv = [pvpsum.tile([128, 512], F32, tag="pv", name=f"pv{h}")
                  for h in range(2)]
            exq = {}

            def pv_mm(kt):
                ex = exq.pop(kt)
                for h in range(2):
                    nc.tensor.matmul(
                        pv[h][0:65, :], lhsT=V65[h][:, kt, 0:65],
                        rhs=ex[:, h * 512:(h + 1) * 512],
                        start=(kt == 0), stop=(kt == NKT - 1),
                    )

            for kt in range(NKT):
                sb = spsum.tile([128, 1024], F32, tag="sb")
                for h in range(2):
                    hsl = slice(h * 64, (h + 1) * 64)
                    nc.tensor.matmul(
                        sb[:, h * 512:(h + 1) * 512],
                        lhsT=QKT[hsl, S + kt * 128:S + (kt + 1) * 128],
                        rhs=QKT[hsl, qsl],
                        start=True, stop=True,
                    )
                ex = expool.tile([128, 1024], F16, tag="ex")
                if kt % 2 == 0 or kt == 1:
                    nc.scalar.activation(
                        ex, sb, mybir.ActivationFunctionType.Exp, scale=0.125)
                else:
                    nc.vector.tensor_scalar(
                        ex[:, :].bitcast(I16), sb[:, :], A16, B16,
                        mybir.AluOpType.mult, mybir.AluOpType.add)
                exq[kt] = ex
                # PV runs 2 slots behind so exp latency stays hidden
                if kt >= 2:
                    pv_mm(kt - 2)
                # spread the previous chunk's out-projection across the loop
                if qc > 0 and kt in (8, 13, 18, 23):
                    outproj_qs(qc - 1, (kt - 8) // 5)
            pv_mm(NKT - 2)
            pv_mm(NKT - 1)
            # normalize: denom rows -> DRAM -> [128, 8] respread -> recip ->
            # DRAM -> partition-broadcast. Avoids 1-lane DVE reciprocal.
            lr = rpool.tile([33, 512], F32, tag="lr", name="lr")
            for h in range(2):
                nc.scalar.copy(lr[32 * h:32 * h + 1, :], pv[h][64:65, :])
            scr = dpool.tile([2, 512], F32, tag="scr")
            nc.sync.dma_start(
                out=scr,
                in_=bass.AP(tensor=lr[:, :].tensor, offset=lr[:, :].offset,
                            ap=[[512 * 32, 2], [1, 512]]),
            )
            rsp = rpool.tile([128, 8], F32, tag="rsp", name="rsp")
            nc.sync.dma_start(
                out=rsp,
                in_=bass.AP(tensor=scr.tensor, offset=scr.offset,
                            ap=[[1, 128], [128, 8]]),
            )
            rsp2 = rpool.tile([128, 8], F32, tag="rsp2", name="rsp2")
            nc.vector.reciprocal(rsp2, rsp)
            scr2 = dpool.tile([2, 512], F32, tag="scr2")
            nc.sync.dma_start(
                out=bass.AP(tensor=scr2.tensor, offset=scr2.offset,
                            ap=[[1, 128], [128, 8]]),
                in_=rsp2,
            )
            bc = rpool.tile([128, 512], F32, tag="bc", name="bc")
            for h in range(2):
                nc.sync.dma_start(
                    out=bc[h * 64:(h + 1) * 64, :],
                    in_=bass.AP(tensor=scr2.tensor, offset=scr2.offset + h * 512,
                                ap=[[0, 64], [1, 512]]),
                )
            for h in range(2):
                nc.vector.tensor_mul(ATT[h * 64:(h + 1) * 64, qsl],
                                     pv[h][0:64, :], bc[h * 64:(h + 1) * 64, :])
        for qs in range(NQS):
            outproj_qs(NSC - 1, qs)
    nc.finalize()
    return nc


def _get_nc(S=SEQ):
    key = S
    if key not in _NC_CACHE:
        _NC_CACHE[key] = _build_nc(S=S)
    return _NC_CACHE[key]


def _make_in_maps(x, Wq, bq, Wk, bk, Wv, bv, Wo):
    xT = np.ascontiguousarray(np.asarray(x, np.float32)[0].T.astype(np.float16))
    Wq, Wk, Wv, Wo = (np.asarray(a, np.float32).astype(np.float16)
                      for a in (Wq, Wk, Wv, Wo))
    bq, bk, bv = (np.asarray(a, np.float32) for a in (bq, bk, bv))
    in_maps = []
    for c in range(N_CORES):
        sl = slice(c * HC, (c + 1) * HC)
        in_maps.append({
            "xT": xT,
            "wq": np.ascontiguousarray(Wq[:, sl]),
            "wk": np.ascontiguousarray(Wk[:, sl]),
            "wv": np.ascontiguousarray(Wv[:, sl]),
            "bq": np.ascontiguousarray(bq[sl]).reshape(HC, 1),
            "bk": np.ascontiguousarray(bk[sl]).reshape(HC, 1),
            "bv": np.ascontiguousarray(bv[sl]).reshape(HC, 1),
            "wo": np.ascontiguousarray(Wo[sl, :]),
        })
    return in_maps


def run(inputs, trace=False):
    """Run the kernel; returns (out [1,S,E] float32, BassKernelResults)."""
    from concourse.bass_utils import run_bass_kernel_spmd

    nc = _get_nc()
    in_maps = _make_in_maps(
        inputs["x"], inputs["Wq"], inputs["bq"], inputs["Wk"], inputs["bk"],
        inputs["Wv"], inputs["bv"], inputs["Wo"],
    )
    res = run_bass_kernel_spmd(
        nc, in_maps, core_ids=list(range(N_CORES)), trace=trace
    )
    acc = np.zeros((SEQ, EMBED), np.float64)
    for c in range(N_CORES):
        acc += res.results[c]["out"].astype(np.float64)
    acc += np.asarray(inputs["bo"], np.float64)
    return acc.astype(np.float32).reshape(1, SEQ, EMBED), res


def kernel(x, Wq, bq, Wk, bk, Wv, bv, Wo, bo):
    out, _ = run(dict(x=x, Wq=Wq, bq=bq, Wk=Wk, bk=bk, Wv=Wv, bv=bv, Wo=Wo, bo=bo))
    return out


# revision 23
# speedup vs baseline: 1.1962x; 1.0427x over previous
"""TRN2 Bass/Tile kernel: 16-head MHA, B=1 S=4096 E=1024, head-sharded over 8 cores.

Sharding: tensor-parallel over heads. Core c owns heads {2c, 2c+1}: columns
[128c, 128(c+1)) of Wq/Wk/Wv (+bias slices) and rows [128c, 128(c+1)) of Wo.
Each core computes attention for its 2 heads and a partial out-projection
[S, E] in fp16; the host sums the 8 partials and adds bo.

v2 pipeline (all fp16 matmuls, fp32 PSUM):
  A) QT/KT [128ch, S] = W^T x (row 0:64 = head0 ch, 64:128 = head1 ch), VT same.
     Bias folded into the PSUM->SBUF evacuation (tensor_scalar_add).
  B) V65[h] [128k, kt, 65] = [V_h | ones] via DMA transpose of VT.
  C) scores^T [k, q]: TWO row-tiled concurrent matmuls (c=64 per head,
     tile_position (0,0)/(64,0)) into one [128, 1024] PSUM pair.
     exp: even kt on ACT (exact, scale=1/8), odd kt on DVE via fp16
     Schraudolph (one tensor_scalar mult+add -> int16 bitcast).
     PV: [65, 512] += V65^T ex accumulated over kt; row 64 = softmax denom.
  D) recip(l) -> DRAM bounce partition-broadcast -> scale -> ATT [128ch, S]
     -> out-proj ATT^T Wo per 128-q tile -> fp16 out.
"""

import sys

for _p in ("/opt/trn_rl_repo", "/opt/pypackages"):
    if _p not in sys.path:
        sys.path.append(_p)

import numpy as np

EMBED = 1024
N_CORES = 8
HC = EMBED // N_CORES  # 128 channels = 2 heads per core
DH = 64                # head dim
SEQ = 4096

# fp16 Schraudolph: exp(s/8) ~= bitcast_f16(int16(A16*s + B16))
A16 = 1024 * 1.4426950408889634 / 8.0
B16 = 15360.0 - 44.25

_NC_CACHE = {}


def _build_nc(S=SEQ, E=EMBED):
    from contextlib import ExitStack

    import concourse.bass as bass
    import concourse.mybir as mybir
    import concourse.tile as tile
    from concourse import bacc

    F32 = mybir.dt.float32
    F16 = mybir.dt.float16
    I16 = mybir.dt.int16

    ET = E // 128      # 8 contraction tiles for projections
    NSC = S // 512     # 8 S-chunks
    NKT = S // 128     # 32 key tiles
    NQS = 512 // 128   # 128-q subtiles per chunk

    nc = bacc.Bacc()
    xT = nc.declare_dram_parameter("xT", [E, S], F16, isOutput=False)
    wq = nc.declare_dram_parameter("wq", [E, HC], F16, isOutput=False)
    wk = nc.declare_dram_parameter("wk", [E, HC], F16, isOutput=False)
    wv = nc.declare_dram_parameter("wv", [E, HC], F16, isOutput=False)
    bq = nc.declare_dram_parameter("bq", [HC, 1], F32, isOutput=False)
    bk = nc.declare_dram_parameter("bk", [HC, 1], F32, isOutput=False)
    bv = nc.declare_dram_parameter("bv", [HC, 1], F32, isOutput=False)
    wo = nc.declare_dram_parameter("wo", [HC, E], F16, isOutput=False)
    out = nc.declare_dram_parameter("out", [S, E], F16, isOutput=True)

    with tile.TileContext(nc) as tc, ExitStack() as ctx:
        wpool = ctx.enter_context(tc.tile_pool(name="w", bufs=1))
        xpool = ctx.enter_context(tc.tile_pool(name="x", bufs=1))
        bigpool = ctx.enter_context(tc.tile_pool(name="big", bufs=1))
        expool = ctx.enter_context(tc.tile_pool(name="e", bufs=4))
        rpool = ctx.enter_context(tc.tile_pool(name="r", bufs=2))
        opool = ctx.enter_context(tc.tile_pool(name="o", bufs=3))
        dpool = ctx.enter_context(tc.tile_pool(name="d", bufs=2, space="DRAM"))
        # PSUM: spsum 2x[128,1024]=4 banks; pv 4x[128,512]=4 banks
        spsum = ctx.enter_context(tc.tile_pool(name="sp", bufs=2, space="PSUM"))
        pvpsum = ctx.enter_context(tc.tile_pool(name="pv", bufs=4, space="PSUM"))

        # --- weights / biases resident ---
        w_sb = {}
        for name, src in (("wq", wq), ("wk", wk), ("wv", wv)):
            t = wpool.tile([128, ET, HC], F16, tag=name, name=name)
            nc.sync.dma_start(out=t, in_=src.rearrange("(a p) c -> p a c", p=128))
            w_sb[name] = t
        wo_sb = wpool.tile([HC, E], F16, tag="wo")
        nc.sync.dma_start(out=wo_sb, in_=wo[:, :])
        b_sb = {}
        for name, src in (("bq", bq), ("bk", bk), ("bv", bv)):
            t = wpool.tile([HC, 1], F32, tag=name, name=name)
            nc.sync.dma_start(out=t, in_=src[:, :])
            b_sb[name] = t

        # --- resident activations ---
        # chunked (sc, et) loads so stage A's first chunk starts ~3us in
        xt = [xpool.tile([128, S], F16, tag=f"xt{et}", name=f"xt{et}")
              for et in range(ET)]
        for sc in range(NSC):
            for et in range(ET):
                nc.sync.dma_start(
                    out=xt[et][:, sc * 512:(sc + 1) * 512],
                    in_=xT[et * 128:(et + 1) * 128, sc * 512:(sc + 1) * 512])
        QKT = bigpool.tile([128, 2 * S], F16, tag="qkt")   # [:, 0:S]=Q, [S:2S]=K
        VT = bigpool.tile([128, S], F16, tag="vt")
        ATT = bigpool.tile([128, S], F16, tag="att")
        # padded to 128 so DMA-transpose outputs land 128-aligned
        V65 = [bigpool.tile([128, NKT, 128], F16, tag=f"v65_{h}", name=f"v65_{h}")
               for h in range(2)]
        for h in range(2):
            nc.vector.memset(V65[h][:, :, 64:65], 1.0)
        ones33 = wpool.tile([33, 64], F32, tag="ones33")
        nc.vector.memset(ones33[0:1, :], 1.0)
        nc.vector.memset(ones33[32:33, :], 1.0)
        # persistent l-row staging; middle rows memset so the full-tile
        # reciprocal below never reads uninitialized data
        lr_t = wpool.tile([33, 512], F32, tag="lr")
        nc.vector.memset(lr_t[:, :], 1.0)
        rcp_t = wpool.tile([33, 512], F32, tag="rcp")

        # --- stage A: projections, 512-wide S chunks ---
        for sc in range(NSC):
            ssl = slice(sc * 512, (sc + 1) * 512)
            big1 = spsum.tile([128, 1024], F32, tag="sb")
            big2 = pvpsum.tile([128, 512], F32, tag="pv")
            for et in range(ET):
                xsl = xt[et][:, ssl]
                first, last = et == 0, et == ET - 1
                nc.tensor.matmul(big1[:, 0:512], lhsT=w_sb["wq"][:, et, :],
                                 rhs=xsl, start=first, stop=last)
                nc.tensor.matmul(big1[:, 512:1024], lhsT=w_sb["wk"][:, et, :],
                                 rhs=xsl, start=first, stop=last)
                nc.tensor.matmul(big2, lhsT=w_sb["wv"][:, et, :],
                                 rhs=xsl, start=first, stop=last)
            nc.vector.tensor_scalar_add(QKT[:, ssl], big1[:, 0:512], b_sb["bq"])
            nc.vector.tensor_scalar_add(QKT[:, S + sc * 512:S + (sc + 1) * 512],
                                        big1[:, 512:1024], b_sb["bk"])
            nc.vector.tensor_scalar_add(VT[:, ssl], big2, b_sb["bv"])
            # stage B: V65 via DMA transpose of freshly-written VT columns
            for h in range(2):
                nc.sync.dma_start_transpose(
                    out=V65[h][:, 4 * sc:4 * sc + 4, 0:64],
                    in_=VT[h * 64:(h + 1) * 64, ssl],
                )

        # --- stage C+D: attention per 512-q chunk; out-proj deferred 1 chunk ---
        def outproj_qs(qq, qs):
            po = spsum.tile([128, 1024], F32, tag="sb", name="po")
            at = ATT[:, qq * 512 + qs * 128:qq * 512 + (qs + 1) * 128]
            nc.tensor.matmul(po[:, 0:512], lhsT=at, rhs=wo_sb[:, 0:512],
                             start=True, stop=True)
            nc.tensor.matmul(po[:, 512:1024], lhsT=at, rhs=wo_sb[:, 512:1024],
                             start=True, stop=True)
            osb = opool.tile([128, 1024], F16, tag="osb", name="osb")
            # alternate the PSUM evacuation engine so neither exp stream stalls
            if qs % 2 == 0:
                nc.scalar.copy(osb, po)
            else:
                nc.vector.tensor_copy(osb, po)
            nc.sync.dma_start(
                out=out[qq * 512 + qs * 128:qq * 512 + (qs + 1) * 128, :],
                in_=osb,
            )

        for qc in range(NSC):
            qsl = slice(qc * 512, (qc + 1) * 512)
            pv = [pvpsum.tile([128, 512], F32, tag="pv", name=f"pv{h}")
                  for h in range(2)]
            exq = {}

            def pv_mm(kt):
                ex = exq.pop(kt)
                for h in range(2):
                    nc.tensor.matmul(
                        pv[h][0:65, :], lhsT=V65[h][:, kt, 0:65],
                        rhs=ex[:, h * 512:(h + 1) * 512],
                        start=(kt == 0), stop=(kt == NKT - 1),
                    )

            for kt in range(NKT):
                sb = spsum.tile([128, 1024], F32, tag="sb")
                for h in range(2):
                    hsl = slice(h * 64, (h + 1) * 64)
                    nc.tensor.matmul(
                        sb[:, h * 512:(h + 1) * 512],
                        lhsT=QKT[hsl, S + kt * 128:S + (kt + 1) * 128],
                        rhs=QKT[hsl, qsl],
                        start=True, stop=True,
                    )
                ex = expool.tile([128, 1024], F16, tag="ex")
                if kt % 2 == 0 or kt == 1:
                    nc.scalar.activation(
                        ex, sb, mybir.ActivationFunctionType.Exp, scale=0.125)
                else:
                    nc.vector.tensor_scalar(
                        ex[:, :].bitcast(I16), sb[:, :], A16, B16,
                        mybir.AluOpType.mult, mybir.AluOpType.add)
                exq[kt] = ex
                # PV runs 2 slots behind so exp latency stays hidden
                if kt >= 2:
                    pv_mm(kt - 2)
                # spread the previous chunk's out-projection across the loop
                if qc > 0 and kt in (16, 20, 24, 28):
                    outproj_qs(qc - 1, (kt - 16) // 4)
            pv_mm(NKT - 2)
            pv_mm(NKT - 1)
            # normalize, all on-chip: l rows -> recip -> PE ones-broadcast ->
            # bc in SBUF -> per-element multiply into ATT.
            for h in range(2):
                nc.scalar.copy(lr_t[32 * h:32 * h + 1, :], pv[h][64:65, :])
            rcp = rcp_t
            nc.vector.reciprocal(rcp, lr_t)
            bcps = pvpsum.tile([128, 512], F32, tag="pv", name="bcps")
            for h in range(2):
                nc.tensor.matmul(bcps[h * 64:(h + 1) * 64, :],
                                 lhsT=ones33[32 * h:32 * h + 1, :],
                                 rhs=rcp[32 * h:32 * h + 1, :],
                                 start=True, stop=True)
            bc = rpool.tile([128, 512], F32, tag="bc", name="bc")
            nc.scalar.copy(bc, bcps)
            for h in range(2):
                nc.vector.tensor_mul(ATT[h * 64:(h + 1) * 64, qsl],
                                     pv[h][0:64, :], bc[h * 64:(h + 1) * 64, :])
        for qs in range(NQS):
            outproj_qs(NSC - 1, qs)
    nc.finalize()
    return nc


def _get_nc(S=SEQ):
    key = S
    if key not in _NC_CACHE:
        _NC_CACHE[key] = _build_nc(S=S)
    return _NC_CACHE[key]


def _make_in_maps(x, Wq, bq, Wk, bk, Wv, bv, Wo):
    xT = np.ascontiguousarray(np.asarray(x, np.float32)[0].T.astype(np.float16))
    Wq, Wk, Wv, Wo = (np.asarray(a, np.float32).astype(np.float16)
                      for a in (Wq, Wk, Wv, Wo))
    bq, bk, bv = (np.asarray(a, np.float32) for a in (bq, bk, bv))
    in_maps = []
    for c in range(N_CORES):
        sl = slice(c * HC, (c + 1) * HC)
        in_maps.append({
            "xT": xT,
            "wq": np.ascontiguousarray(Wq[:, sl]),
            "wk": np.ascontiguousarray(Wk[:, sl]),
            "wv": np.ascontiguousarray(Wv[:, sl]),
            "bq": np.ascontiguousarray(bq[sl]).reshape(HC, 1),
            "bk": np.ascontiguousarray(bk[sl]).reshape(HC, 1),
            "bv": np.ascontiguousarray(bv[sl]).reshape(HC, 1),
            "wo": np.ascontiguousarray(Wo[sl, :]),
        })
    return in_maps


def run(inputs, trace=False):
    """Run the kernel; returns (out [1,S,E] float32, BassKernelResults)."""
    from concourse.bass_utils import run_bass_kernel_spmd

    nc = _get_nc()
    in_maps = _make_in_maps(
        inputs["x"], inputs["Wq"], inputs["bq"], inputs["Wk"], inputs["bk"],
        inputs["Wv"], inputs["bv"], inputs["Wo"],
    )
    res = run_bass_kernel_spmd(
        nc, in_maps, core_ids=list(range(N_CORES)), trace=trace
    )
    acc = np.zeros((SEQ, EMBED), np.float64)
    for c in range(N_CORES):
        acc += res.results[c]["out"].astype(np.float64)
    acc += np.asarray(inputs["bo"], np.float64)
    return acc.astype(np.float32).reshape(1, SEQ, EMBED), res


def kernel(x, Wq, bq, Wk, bk, Wv, bv, Wo, bo):
    out, _ = run(dict(x=x, Wq=Wq, bq=bq, Wk=Wk, bk=bk, Wv=Wv, bv=bv, Wo=Wo, bo=bo))
    return out


# revision 28
# speedup vs baseline: 1.4285x; 1.1942x over previous
"""TRN2 Bass/Tile kernel: 16-head MHA, B=1 S=4096 E=1024, head-sharded over 8 cores.

Sharding: tensor-parallel over heads. Core c owns heads {2c, 2c+1}: columns
[128c, 128(c+1)) of Wq/Wk/Wv (+bias slices) and rows [128c, 128(c+1)) of Wo.
Each core computes attention for its 2 heads and a partial out-projection
[S, E] in fp16; the host sums the 8 partials and adds bo.

v2 pipeline (all fp16 matmuls, fp32 PSUM):
  A) QT/KT [128ch, S] = W^T x (row 0:64 = head0 ch, 64:128 = head1 ch), VT same.
     Bias folded into the PSUM->SBUF evacuation (tensor_scalar_add).
  B) V65[h] [128k, kt, 65] = [V_h | ones] via DMA transpose of VT.
  C) scores^T [k, q]: TWO row-tiled concurrent matmuls (c=64 per head,
     tile_position (0,0)/(64,0)) into one [128, 1024] PSUM pair.
     exp: even kt on ACT (exact, scale=1/8), odd kt on DVE via fp16
     Schraudolph (one tensor_scalar mult+add -> int16 bitcast).
     PV: [65, 512] += V65^T ex accumulated over kt; row 64 = softmax denom.
  D) recip(l) -> DRAM bounce partition-broadcast -> scale -> ATT [128ch, S]
     -> out-proj ATT^T Wo per 128-q tile -> fp16 out.
"""

import sys

for _p in ("/opt/trn_rl_repo", "/opt/pypackages"):
    if _p not in sys.path:
        sys.path.append(_p)

import numpy as np

EMBED = 1024
N_CORES = 8
HC = EMBED // N_CORES  # 128 channels = 2 heads per core
DH = 64                # head dim
SEQ = 4096

# fp16 Schraudolph: exp(s/8) ~= bitcast_f16(int16(A16*s + B16))
A16 = 1024 * 1.4426950408889634 / 8.0
B16 = 15360.0 - 44.25

_NC_CACHE = {}


def _build_nc(S=SEQ, E=EMBED):
    from contextlib import ExitStack

    import concourse.bass as bass
    import concourse.mybir as mybir
    import concourse.tile as tile
    from concourse import bacc

    F32 = mybir.dt.float32
    F16 = mybir.dt.float16
    I16 = mybir.dt.int16

    ET = E // 128      # 8 contraction tiles for projections
    NSC = S // 512     # 8 S-chunks
    NKT = S // 128     # 32 key tiles
    NQS = 512 // 128   # 128-q subtiles per chunk

    nc = bacc.Bacc()
    xT = nc.declare_dram_parameter("xT", [E, S], F16, isOutput=False)
    wq = nc.declare_dram_parameter("wq", [E, HC], F16, isOutput=False)
    wk = nc.declare_dram_parameter("wk", [E, HC], F16, isOutput=False)
    wv = nc.declare_dram_parameter("wv", [E, HC], F16, isOutput=False)
    bq = nc.declare_dram_parameter("bq", [HC, 1], F32, isOutput=False)
    bk = nc.declare_dram_parameter("bk", [HC, 1], F32, isOutput=False)
    bv = nc.declare_dram_parameter("bv", [HC, 1], F32, isOutput=False)
    wo = nc.declare_dram_parameter("wo", [HC, E], F16, isOutput=False)
    out = nc.declare_dram_parameter("out", [S, E], F16, isOutput=True)

    with tile.TileContext(nc) as tc, ExitStack() as ctx:
        wpool = ctx.enter_context(tc.tile_pool(name="w", bufs=1))
        xpool = ctx.enter_context(tc.tile_pool(name="x", bufs=1))
        bigpool = ctx.enter_context(tc.tile_pool(name="big", bufs=1))
        expool = ctx.enter_context(tc.tile_pool(name="e", bufs=4))
        rpool = ctx.enter_context(tc.tile_pool(name="r", bufs=2))
        opool = ctx.enter_context(tc.tile_pool(name="o", bufs=3))
        dpool = ctx.enter_context(tc.tile_pool(name="d", bufs=2, space="DRAM"))
        # PSUM: spsum 2x[128,1024]=4 banks; pv 4x[128,512]=4 banks
        spsum = ctx.enter_context(tc.tile_pool(name="sp", bufs=2, space="PSUM"))
        pvpsum = ctx.enter_context(tc.tile_pool(name="pv", bufs=4, space="PSUM"))

        # --- weights / biases resident ---
        w_sb = {}
        for name, src in (("wq", wq), ("wk", wk), ("wv", wv)):
            t = wpool.tile([128, ET, HC], F16, tag=name, name=name)
            nc.sync.dma_start(out=t, in_=src.rearrange("(a p) c -> p a c", p=128))
            w_sb[name] = t
        wo_sb = wpool.tile([HC, E], F16, tag="wo")
        nc.sync.dma_start(out=wo_sb, in_=wo[:, :])
        b_sb = {}
        for name, src in (("bq", bq), ("bk", bk), ("bv", bv)):
            t = wpool.tile([HC, 1], F32, tag=name, name=name)
            nc.sync.dma_start(out=t, in_=src[:, :])
            b_sb[name] = t

        # --- resident activations ---
        # chunked (sc, et) loads so stage A's first chunk starts ~3us in
        xt = [xpool.tile([128, S], F16, tag=f"xt{et}", name=f"xt{et}")
              for et in range(ET)]
        for sc in range(NSC):
            for et in range(ET):
                nc.sync.dma_start(
                    out=xt[et][:, sc * 512:(sc + 1) * 512],
                    in_=xT[et * 128:(et + 1) * 128, sc * 512:(sc + 1) * 512])
        QKT = bigpool.tile([128, 2 * S], F16, tag="qkt")   # [:, 0:S]=Q, [S:2S]=K
        VT = bigpool.tile([128, S], F16, tag="vt")
        ATT = bigpool.tile([128, S], F16, tag="att")
        # padded to 128 so DMA-transpose outputs land 128-aligned
        V65 = [bigpool.tile([128, NKT, 128], F16, tag=f"v65_{h}", name=f"v65_{h}")
               for h in range(2)]
        for h in range(2):
            nc.vector.memset(V65[h][:, :, 64:65], 1.0)
        ones33 = wpool.tile([33, 64], F32, tag="ones33")
        nc.vector.memset(ones33[0:1, :], 1.0)
        nc.vector.memset(ones33[32:33, :], 1.0)
        # persistent l-row staging; middle rows memset so the full-tile
        # reciprocal below never reads uninitialized data
        lr_t = wpool.tile([33, 512], F32, tag="lr")
        nc.vector.memset(lr_t[:, :], 1.0)
        rcp_t = wpool.tile([33, 512], F32, tag="rcp")

        # --- stage A: projections, 512-wide S chunks ---
        for sc in range(NSC):
            ssl = slice(sc * 512, (sc + 1) * 512)
            big1 = spsum.tile([128, 1024], F32, tag="sb")
            big2 = pvpsum.tile([128, 512], F32, tag="pv")
            for et in range(ET):
                xsl = xt[et][:, ssl]
                first, last = et == 0, et == ET - 1
                nc.tensor.matmul(big1[:, 0:512], lhsT=w_sb["wq"][:, et, :],
                                 rhs=xsl, start=first, stop=last)
                nc.tensor.matmul(big1[:, 512:1024], lhsT=w_sb["wk"][:, et, :],
                                 rhs=xsl, start=first, stop=last)
                nc.tensor.matmul(big2, lhsT=w_sb["wv"][:, et, :],
                                 rhs=xsl, start=first, stop=last)
            nc.vector.tensor_scalar_add(QKT[:, ssl], big1[:, 0:512], b_sb["bq"])
            nc.vector.tensor_scalar_add(QKT[:, S + sc * 512:S + (sc + 1) * 512],
                                        big1[:, 512:1024], b_sb["bk"])
            nc.vector.tensor_scalar_add(VT[:, ssl], big2, b_sb["bv"])
            # stage B: V65 via DMA transpose of freshly-written VT columns
            for h in range(2):
                nc.sync.dma_start_transpose(
                    out=V65[h][:, 4 * sc:4 * sc + 4, 0:64],
                    in_=VT[h * 64:(h + 1) * 64, ssl],
                )

        # --- stage C+D: attention per 512-q chunk; out-proj deferred 1 chunk ---
        def outproj_qs(qq, qs):
            po = spsum.tile([128, 1024], F32, tag="sb", name="po")
            at = ATT[:, qq * 512 + qs * 128:qq * 512 + (qs + 1) * 128]
            nc.tensor.matmul(po[:, 0:512], lhsT=at, rhs=wo_sb[:, 0:512],
                             start=True, stop=True)
            nc.tensor.matmul(po[:, 512:1024], lhsT=at, rhs=wo_sb[:, 512:1024],
                             start=True, stop=True)
            osb = opool.tile([128, 1024], F16, tag="osb", name="osb")
            # alternate the PSUM evacuation engine so neither exp stream stalls
            if qs % 2 == 0:
                nc.scalar.copy(osb, po)
            else:
                nc.vector.tensor_copy(osb, po)
            nc.sync.dma_start(
                out=out[qq * 512 + qs * 128:qq * 512 + (qs + 1) * 128, :],
                in_=osb,
            )

        pv_prev = None

        def normalize_step(qq, pvq, step):
            qsl2 = slice(qq * 512, (qq + 1) * 512)
            if step == 0:
                for h in range(2):
                    nc.scalar.copy(lr_t[32 * h:32 * h + 1, :], pvq[h][64:65, :])
            elif step == 1:
                nc.vector.reciprocal(rcp_t, lr_t)
            elif step == 2:
                bcps = spsum.tile([128, 1024], F32, tag="sb", name="bcps")
                for h in range(2):
                    nc.tensor.matmul(bcps[h * 64:(h + 1) * 64, 0:512],
                                     lhsT=ones33[32 * h:32 * h + 1, :],
                                     rhs=rcp_t[32 * h:32 * h + 1, :],
                                     start=True, stop=True)
                bc = rpool.tile([128, 512], F32, tag="bc", name="bc")
                nc.scalar.copy(bc, bcps[:, 0:512])
                normalize_step.bc = bc
            else:
                for h in range(2):
                    nc.vector.tensor_mul(
                        ATT[h * 64:(h + 1) * 64, qsl2], pvq[h][0:64, :],
                        normalize_step.bc[h * 64:(h + 1) * 64, :])

        for qc in range(NSC):
            qsl = slice(qc * 512, (qc + 1) * 512)
            pv = [pvpsum.tile([128, 512], F32, tag="pv", name=f"pv{h}")
                  for h in range(2)]
            exq = {}

            def pv_mm(kt):
                ex = exq.pop(kt)
                for h in range(2):
                    nc.tensor.matmul(
                        pv[h][0:65, :], lhsT=V65[h][:, kt, 0:65],
                        rhs=ex[:, h * 512:(h + 1) * 512],
                        start=(kt == 0), stop=(kt == NKT - 1),
                    )

            for kt in range(NKT):
                sb = spsum.tile([128, 1024], F32, tag="sb")
                for h in range(2):
                    hsl = slice(h * 64, (h + 1) * 64)
                    nc.tensor.matmul(
                        sb[:, h * 512:(h + 1) * 512],
                        lhsT=QKT[hsl, S + kt * 128:S + (kt + 1) * 128],
                        rhs=QKT[hsl, qsl],
                        start=True, stop=True,
                    )
                ex = expool.tile([128, 1024], F16, tag="ex")
                if kt % 2 == 0 or kt in (1, 3):
                    nc.scalar.activation(
                        ex, sb, mybir.ActivationFunctionType.Exp, scale=0.125)
                else:
                    nc.vector.tensor_scalar(
                        ex[:, :].bitcast(I16), sb[:, :], A16, B16,
                        mybir.AluOpType.mult, mybir.AluOpType.add)
                exq[kt] = ex
                # PV runs 2 slots behind so exp latency stays hidden
                if kt >= 2:
                    pv_mm(kt - 2)
                # previous chunk's normalize + out-projection, spread across
                # this chunk's slots so they never precede our score matmuls
                # in any engine stream
                if pv_prev is not None:
                    if 2 <= kt <= 5:
                        normalize_step(qc - 1, pv_prev, kt - 2)
                    elif kt in (16, 20, 24, 28):
                        outproj_qs(qc - 1, (kt - 16) // 4)
            pv_mm(NKT - 2)
            pv_mm(NKT - 1)
            pv_prev = pv
        # tail: last chunk's normalize + out-projection
        for step in range(4):
            normalize_step(NSC - 1, pv_prev, step)
        for qs in range(NQS):
            outproj_qs(NSC - 1, qs)
    nc.finalize()
    return nc


def _get_nc(S=SEQ):
    key = S
    if key not in _NC_CACHE:
        _NC_CACHE[key] = _build_nc(S=S)
    return _NC_CACHE[key]


def _make_in_maps(x, Wq, bq, Wk, bk, Wv, bv, Wo):
    xT = np.ascontiguousarray(np.asarray(x, np.float32)[0].T.astype(np.float16))
    Wq, Wk, Wv, Wo = (np.asarray(a, np.float32).astype(np.float16)
                      for a in (Wq, Wk, Wv, Wo))
    bq, bk, bv = (np.asarray(a, np.float32) for a in (bq, bk, bv))
    in_maps = []
    for c in range(N_CORES):
        sl = slice(c * HC, (c + 1) * HC)
        in_maps.append({
            "xT": xT,
            "wq": np.ascontiguousarray(Wq[:, sl]),
            "wk": np.ascontiguousarray(Wk[:, sl]),
            "wv": np.ascontiguousarray(Wv[:, sl]),
            "bq": np.ascontiguousarray(bq[sl]).reshape(HC, 1),
            "bk": np.ascontiguousarray(bk[sl]).reshape(HC, 1),
            "bv": np.ascontiguousarray(bv[sl]).reshape(HC, 1),
            "wo": np.ascontiguousarray(Wo[sl, :]),
        })
    return in_maps


def run(inputs, trace=False):
    """Run the kernel; returns (out [1,S,E] float32, BassKernelResults)."""
    from concourse.bass_utils import run_bass_kernel_spmd

    nc = _get_nc()
    in_maps = _make_in_maps(
        inputs["x"], inputs["Wq"], inputs["bq"], inputs["Wk"], inputs["bk"],
        inputs["Wv"], inputs["bv"], inputs["Wo"],
    )
    res = run_bass_kernel_spmd(
        nc, in_maps, core_ids=list(range(N_CORES)), trace=trace
    )
    acc = np.zeros((SEQ, EMBED), np.float64)
    for c in range(N_CORES):
        acc += res.results[c]["out"].astype(np.float64)
    acc += np.asarray(inputs["bo"], np.float64)
    return acc.astype(np.float32).reshape(1, SEQ, EMBED), res


def kernel(x, Wq, bq, Wk, bk, Wv, bv, Wo, bo):
    out, _ = run(dict(x=x, Wq=Wq, bq=bq, Wk=Wk, bk=bk, Wv=Wv, bv=bv, Wo=Wo, bo=bo))
    return out


# revision 38
# speedup vs baseline: 1.4529x; 1.0171x over previous
"""TRN2 Bass/Tile kernel: 16-head MHA, B=1 S=4096 E=1024, head-sharded over 8 cores.

Sharding: tensor-parallel over heads. Core c owns heads {2c, 2c+1}: columns
[128c, 128(c+1)) of Wq/Wk/Wv (+bias slices) and rows [128c, 128(c+1)) of Wo.
Each core computes attention for its 2 heads and a partial out-projection
[S, E] in fp16; the host sums the 8 partials and adds bo.

v2 pipeline (all fp16 matmuls, fp32 PSUM):
  A) QT/KT [128ch, S] = W^T x (row 0:64 = head0 ch, 64:128 = head1 ch), VT same.
     Bias folded into the PSUM->SBUF evacuation (tensor_scalar_add).
  B) V65[h] [128k, kt, 65] = [V_h | ones] via DMA transpose of VT.
  C) scores^T [k, q]: TWO row-tiled concurrent matmuls (c=64 per head,
     tile_position (0,0)/(64,0)) into one [128, 1024] PSUM pair.
     exp: even kt on ACT (exact, scale=1/8), odd kt on DVE via fp16
     Schraudolph (one tensor_scalar mult+add -> int16 bitcast).
     PV: [65, 512] += V65^T ex accumulated over kt; row 64 = softmax denom.
  D) recip(l) -> DRAM bounce partition-broadcast -> scale -> ATT [128ch, S]
     -> out-proj ATT^T Wo per 128-q tile -> fp16 out.
"""

import sys

for _p in ("/opt/trn_rl_repo", "/opt/pypackages"):
    if _p not in sys.path:
        sys.path.append(_p)

import numpy as np

EMBED = 1024
N_CORES = 8
HC = EMBED // N_CORES  # 128 channels = 2 heads per core
DH = 64                # head dim
SEQ = 4096

# fp16 Schraudolph: exp(s/8) ~= bitcast_f16(int16(A16*s + B16))
A16 = 1024 * 1.4426950408889634 / 8.0
B16 = 15360.0 - 44.25

_NC_CACHE = {}


def _build_nc(S=SEQ, E=EMBED):
    from contextlib import ExitStack

    import concourse.bass as bass
    import concourse.mybir as mybir
    import concourse.tile as tile
    from concourse import bacc

    F32 = mybir.dt.float32
    F16 = mybir.dt.float16
    I16 = mybir.dt.int16

    ET = E // 128      # 8 contraction tiles for projections
    NSC = S // 512     # 8 S-chunks
    NKT = S // 128     # 32 key tiles
    NQS = 512 // 128   # 128-q subtiles per chunk

    nc = bacc.Bacc()
    xT = nc.declare_dram_parameter("xT", [E, S], F16, isOutput=False)
    # host pre-rearranged to [128, ET*HC] so each partition is one 2KB line
    wq = nc.declare_dram_parameter("wq", [128, ET * HC], F16, isOutput=False)
    wk = nc.declare_dram_parameter("wk", [128, ET * HC], F16, isOutput=False)
    wv = nc.declare_dram_parameter("wv", [128, ET * HC], F16, isOutput=False)
    bq = nc.declare_dram_parameter("bq", [HC, 1], F32, isOutput=False)
    bk = nc.declare_dram_parameter("bk", [HC, 1], F32, isOutput=False)
    bv = nc.declare_dram_parameter("bv", [HC, 1], F32, isOutput=False)
    wo = nc.declare_dram_parameter("wo", [HC, E], F16, isOutput=False)
    out = nc.declare_dram_parameter("out", [S, E], F16, isOutput=True)

    with tile.TileContext(nc) as tc, ExitStack() as ctx:
        wpool = ctx.enter_context(tc.tile_pool(name="w", bufs=1))
        xpool = ctx.enter_context(tc.tile_pool(name="x", bufs=1))
        bigpool = ctx.enter_context(tc.tile_pool(name="big", bufs=1))
        expool = ctx.enter_context(tc.tile_pool(name="e", bufs=4))
        rpool = ctx.enter_context(tc.tile_pool(name="r", bufs=2))
        opool = ctx.enter_context(tc.tile_pool(name="o", bufs=3))
        dpool = ctx.enter_context(tc.tile_pool(name="d", bufs=2, space="DRAM"))
        # PSUM: spsum 2x[128,1024]=4 banks; pv 4x[128,512]=4 banks
        spsum = ctx.enter_context(tc.tile_pool(name="sp", bufs=2, space="PSUM"))
        pvpsum = ctx.enter_context(tc.tile_pool(name="pv", bufs=4, space="PSUM"))

        # --- weights / biases / x loads; first S-chunk's x + weights first ---
        xt = [xpool.tile([128, S], F16, tag=f"xt{et}", name=f"xt{et}")
              for et in range(ET)]
        for et in range(ET):
            nc.sync.dma_start(
                out=xt[et][:, 0:512], in_=xT[et * 128:(et + 1) * 128, 0:512])
        w_sb = {}
        for name, src in (("wq", wq), ("wk", wk), ("wv", wv)):
            t = wpool.tile([128, ET, HC], F16, tag=name, name=name)
            nc.sync.dma_start(out=t, in_=src.rearrange("p (a c) -> p a c", c=HC))
            w_sb[name] = t
        b_sb = {}
        for name, src in (("bq", bq), ("bk", bk), ("bv", bv)):
            t = wpool.tile([HC, 1], F32, tag=name, name=name)
            nc.sync.dma_start(out=t, in_=src[:, :])
            b_sb[name] = t
        for sc in range(1, NSC):
            for et in range(ET):
                nc.sync.dma_start(
                    out=xt[et][:, sc * 512:(sc + 1) * 512],
                    in_=xT[et * 128:(et + 1) * 128, sc * 512:(sc + 1) * 512])
        wo_sb = wpool.tile([HC, E], F16, tag="wo")
        nc.sync.dma_start(out=wo_sb, in_=wo[:, :])
        QKT = bigpool.tile([128, 2 * S], F16, tag="qkt")   # [:, 0:S]=Q, [S:2S]=K
        VT = bigpool.tile([128, S], F16, tag="vt")
        ATT = bigpool.tile([128, S], F16, tag="att")
        # padded to 128 so DMA-transpose outputs land 128-aligned
        V65 = [bigpool.tile([128, NKT, 128], F16, tag=f"v65_{h}", name=f"v65_{h}")
               for h in range(2)]
        for h in range(2):
            nc.vector.memset(V65[h][:, :, 64:65], 1.0)
        ones33 = wpool.tile([33, 64], F16, tag="ones33")
        nc.vector.memset(ones33[0:1, :], 1.0)
        nc.vector.memset(ones33[32:33, :], 1.0)
        # persistent l-row staging (fp16, carrying l/64 so 1/x stays normal);
        # middle rows memset so the full-tile reciprocal reads no garbage
        lr_t = wpool.tile([33, 512], F16, tag="lr")
        nc.vector.memset(lr_t[:, :], 1.0)
        rcp_t = wpool.tile([33, 512], F16, tag="rcp")

        # --- stage A: projections, 512-wide S chunks ---
        for sc in range(NSC):
            ssl = slice(sc * 512, (sc + 1) * 512)
            big1 = spsum.tile([128, 1024], F32, tag="sb")
            big2 = pvpsum.tile([128, 512], F32, tag="pv")
            for et in range(ET):
                xsl = xt[et][:, ssl]
                first, last = et == 0, et == ET - 1
                nc.tensor.matmul(big1[:, 0:512], lhsT=w_sb["wq"][:, et, :],
                                 rhs=xsl, start=first, stop=last)
                nc.tensor.matmul(big1[:, 512:1024], lhsT=w_sb["wk"][:, et, :],
                                 rhs=xsl, start=first, stop=last)
                nc.tensor.matmul(big2, lhsT=w_sb["wv"][:, et, :],
                                 rhs=xsl, start=first, stop=last)
            nc.vector.tensor_scalar_add(QKT[:, ssl], big1[:, 0:512], b_sb["bq"])
            nc.vector.tensor_scalar_add(QKT[:, S + sc * 512:S + (sc + 1) * 512],
                                        big1[:, 512:1024], b_sb["bk"])
            nc.vector.tensor_scalar_add(VT[:, ssl], big2, b_sb["bv"])
            # stage B: V65 via DMA transpose of freshly-written VT columns
            for h in range(2):
                nc.sync.dma_start_transpose(
                    out=V65[h][:, 4 * sc:4 * sc + 4, 0:64],
                    in_=VT[h * 64:(h + 1) * 64, ssl],
                )

        # --- stage C+D: attention per 512-q chunk; out-proj deferred 1 chunk ---
        def outproj_qs(qq, qs):
            po = spsum.tile([128, 1024], F32, tag="sb", name="po")
            at = ATT[:, qq * 512 + qs * 128:qq * 512 + (qs + 1) * 128]
            nc.tensor.matmul(po[:, 0:512], lhsT=at, rhs=wo_sb[:, 0:512],
                             start=True, stop=True)
            nc.tensor.matmul(po[:, 512:1024], lhsT=at, rhs=wo_sb[:, 512:1024],
                             start=True, stop=True)
            osb = opool.tile([128, 1024], F16, tag="osb", name="osb")
            # alternate the PSUM evacuation engine so neither exp stream stalls
            if qs % 2 == 0:
                nc.scalar.copy(osb, po)
            else:
                nc.vector.tensor_copy(osb, po)
            nc.sync.dma_start(
                out=out[qq * 512 + qs * 128:qq * 512 + (qs + 1) * 128, :],
                in_=osb,
            )

        pv_prev = None
        ns_state = {}

        def normalize_step(qq, pvq, step):
            qsl2 = slice(qq * 512, (qq + 1) * 512)
            if step == 0:
                # evacuate pv (scaled 1/64) to SBUF fp16, freeing PSUM banks;
                # both heads stacked so the final multiply is one op
                pvc = rpool.tile([128, 512], F16, tag="pvc", name="pvc")
                for h in range(2):
                    nc.scalar.mul(pvc[h * 64:(h + 1) * 64, :],
                                  pvq[h][0:64, :], 1.0 / 64.0)
                ns_state["pvc"] = pvc
            elif step == 1:
                for h in range(2):
                    nc.scalar.mul(lr_t[32 * h:32 * h + 1, :], pvq[h][64:65, :],
                                  1.0 / 64.0)
            elif step == 2:
                with nc.allow_low_precision(
                        reason="fp16 1/l; 5e-4 rel vs 2e-2 budget"):
                    nc.vector.reciprocal(rcp_t, lr_t)
            elif step == 3:
                # PE broadcast of the 64/l rows across partitions
                bcps = pvpsum.tile([128, 512], F32, tag="pv", name="bcps")
                for h in range(2):
                    nc.tensor.matmul(bcps[h * 64:(h + 1) * 64, :],
                                     lhsT=ones33[32 * h:32 * h + 1, :],
                                     rhs=rcp_t[32 * h:32 * h + 1, :],
                                     start=True, stop=True)
                bc = rpool.tile([128, 512], F16, tag="bc", name="bc")
                nc.scalar.copy(bc, bcps)
                ns_state["bc"] = bc
            else:
                nc.vector.tensor_mul(ATT[:, qsl2], ns_state["pvc"][:, :],
                                     ns_state["bc"][:, :])

        for qc in range(NSC):
            qsl = slice(qc * 512, (qc + 1) * 512)
            pv = [pvpsum.tile([128, 512], F32, tag="pv", name=f"pv{h}")
                  for h in range(2)]
            exq = {}

            def pv_mm(kt):
                ex = exq.pop(kt)
                for h in range(2):
                    nc.tensor.matmul(
                        pv[h][0:65, :], lhsT=V65[h][:, kt, 0:65],
                        rhs=ex[:, h * 512:(h + 1) * 512],
                        start=(kt == 0), stop=(kt == NKT - 1),
                    )

            for kt in range(NKT):
                sb = spsum.tile([128, 1024], F32, tag="sb")
                for h in range(2):
                    hsl = slice(h * 64, (h + 1) * 64)
                    nc.tensor.matmul(
                        sb[:, h * 512:(h + 1) * 512],
                        lhsT=QKT[hsl, S + kt * 128:S + (kt + 1) * 128],
                        rhs=QKT[hsl, qsl],
                        start=True, stop=True,
                    )
                ex = expool.tile([128, 1024], F16, tag="ex")
                if kt % 2 == 0 or kt in (1, 3):
                    nc.scalar.activation(
                        ex, sb, mybir.ActivationFunctionType.Exp, scale=0.125)
                else:
                    nc.vector.tensor_scalar(
                        ex[:, :].bitcast(I16), sb[:, :], A16, B16,
                        mybir.AluOpType.mult, mybir.AluOpType.add)
                exq[kt] = ex
                # PV runs 2 slots behind so exp latency stays hidden
                if kt >= 2:
                    pv_mm(kt - 2)
                # previous chunk's normalize + out-projection, spread across
                # this chunk's slots so they never precede our score matmuls
                # in any engine stream
                if pv_prev is not None:
                    if kt in (2, 3, 4, 8, 10):
                        normalize_step(qc - 1, pv_prev,
                                       {2: 0, 3: 1, 4: 2, 8: 3, 10: 4}[kt])
                    elif kt in (16, 20, 24, 28):
                        outproj_qs(qc - 1, (kt - 16) // 4)
            pv_mm(NKT - 2)
            pv_mm(NKT - 1)
            pv_prev = pv
        # tail: last chunk's normalize + out-projection
        for step in range(5):
            normalize_step(NSC - 1, pv_prev, step)
        for qs in range(NQS):
            outproj_qs(NSC - 1, qs)
    nc.finalize()
    return nc


def _get_nc(S=SEQ):
    key = S
    if key not in _NC_CACHE:
        _NC_CACHE[key] = _build_nc(S=S)
    return _NC_CACHE[key]


def _rearr_w(w):
    # [E, HC] -> [128, ET*HC]: w_r[p, a*HC + c] = w[a*128 + p, c]
    et = EMBED // 128
    return np.ascontiguousarray(
        w.reshape(et, 128, HC).transpose(1, 0, 2).reshape(128, et * HC))


def _make_in_maps(x, Wq, bq, Wk, bk, Wv, bv, Wo):
    xT = np.ascontiguousarray(np.asarray(x, np.float32)[0].T.astype(np.float16))
    Wq, Wk, Wv, Wo = (np.asarray(a, np.float32).astype(np.float16)
                      for a in (Wq, Wk, Wv, Wo))
    bq, bk, bv = (np.asarray(a, np.float32) for a in (bq, bk, bv))
    in_maps = []
    for c in range(N_CORES):
        sl = slice(c * HC, (c + 1) * HC)
        in_maps.append({
            "xT": xT,
            "wq": _rearr_w(Wq[:, sl]),
            "wk": _rearr_w(Wk[:, sl]),
            "wv": _rearr_w(Wv[:, sl]),
            "bq": np.ascontiguousarray(bq[sl]).reshape(HC, 1),
            "bk": np.ascontiguousarray(bk[sl]).reshape(HC, 1),
            "bv": np.ascontiguousarray(bv[sl]).reshape(HC, 1),
            "wo": np.ascontiguousarray(Wo[sl, :]),
        })
    return in_maps


def run(inputs, trace=False):
    """Run the kernel; returns (out [1,S,E] float32, BassKernelResults)."""
    from concourse.bass_utils import run_bass_kernel_spmd

    nc = _get_nc()
    in_maps = _make_in_maps(
        inputs["x"], inputs["Wq"], inputs["bq"], inputs["Wk"], inputs["bk"],
        inputs["Wv"], inputs["bv"], inputs["Wo"],
    )
    res = run_bass_kernel_spmd(
        nc, in_maps, core_ids=list(range(N_CORES)), trace=trace
    )
    acc = np.zeros((SEQ, EMBED), np.float64)
    for c in range(N_CORES):
        acc += res.results[c]["out"].astype(np.float64)
    acc += np.asarray(inputs["bo"], np.float64)
    return acc.astype(np.float32).reshape(1, SEQ, EMBED), res


def kernel(x, Wq, bq, Wk, bk, Wv, bv, Wo, bo):
    out, _ = run(dict(x=x, Wq=Wq, bq=bq, Wk=Wk, bk=bk, Wv=Wv, bv=bv, Wo=Wo, bo=bo))
    return out


# revision 40
# speedup vs baseline: 1.6338x; 1.1245x over previous
"""TRN2 Bass/Tile kernel: 16-head MHA, B=1 S=4096 E=1024, head-sharded over 8 cores.

Sharding: tensor-parallel over heads. Core c owns heads {2c, 2c+1}: columns
[128c, 128(c+1)) of Wq/Wk/Wv (+bias slices) and rows [128c, 128(c+1)) of Wo.
Each core computes attention for its 2 heads and a partial out-projection
[S, E] in fp16; the host sums the 8 partials and adds bo.

v2 pipeline (all fp16 matmuls, fp32 PSUM):
  A) QT/KT [128ch, S] = W^T x (row 0:64 = head0 ch, 64:128 = head1 ch), VT same.
     Bias folded into the PSUM->SBUF evacuation (tensor_scalar_add).
  B) V65[h] [128k, kt, 65] = [V_h | ones] via DMA transpose of VT.
  C) scores^T [k, q]: TWO row-tiled concurrent matmuls (c=64 per head,
     tile_position (0,0)/(64,0)) into one [128, 1024] PSUM pair.
     exp: even kt on ACT (exact, scale=1/8), odd kt on DVE via fp16
     Schraudolph (one tensor_scalar mult+add -> int16 bitcast).
     PV: [65, 512] += V65^T ex accumulated over kt; row 64 = softmax denom.
  D) recip(l) -> DRAM bounce partition-broadcast -> scale -> ATT [128ch, S]
     -> out-proj ATT^T Wo per 128-q tile -> fp16 out.
"""

import sys

for _p in ("/opt/trn_rl_repo", "/opt/pypackages"):
    if _p not in sys.path:
        sys.path.append(_p)

import numpy as np

EMBED = 1024
N_CORES = 8
HC = EMBED // N_CORES  # 128 channels = 2 heads per core
DH = 64                # head dim
SEQ = 4096

# fp16 Schraudolph: exp(s/8) ~= bitcast_f16(int16(A16*s + B16))
A16 = 1024 * 1.4426950408889634 / 8.0
B16 = 15360.0 - 44.25

_NC_CACHE = {}


def _build_nc(S=SEQ, E=EMBED):
    from contextlib import ExitStack

    import concourse.bass as bass
    import concourse.mybir as mybir
    import concourse.tile as tile
    from concourse import bacc

    F32 = mybir.dt.float32
    F16 = mybir.dt.float16
    I16 = mybir.dt.int16

    ET = E // 128      # 8 contraction tiles for projections
    NSC = S // 512     # 8 S-chunks
    NKT = S // 128     # 32 key tiles
    NQS = 512 // 128   # 128-q subtiles per chunk

    nc = bacc.Bacc()
    xT = nc.declare_dram_parameter("xT", [E, S], F16, isOutput=False)
    # host pre-rearranged to [128, ET*HC] so each partition is one 2KB line
    wq = nc.declare_dram_parameter("wq", [128, ET * HC], F16, isOutput=False)
    wk = nc.declare_dram_parameter("wk", [128, ET * HC], F16, isOutput=False)
    wv = nc.declare_dram_parameter("wv", [128, ET * HC], F16, isOutput=False)
    bq = nc.declare_dram_parameter("bq", [HC, 1], F32, isOutput=False)
    bk = nc.declare_dram_parameter("bk", [HC, 1], F32, isOutput=False)
    bv = nc.declare_dram_parameter("bv", [HC, 1], F32, isOutput=False)
    wo = nc.declare_dram_parameter("wo", [HC, E], F16, isOutput=False)
    out = nc.declare_dram_parameter("out", [S, E], F16, isOutput=True)

    with tile.TileContext(nc) as tc, ExitStack() as ctx:
        wpool = ctx.enter_context(tc.tile_pool(name="w", bufs=1))
        xpool = ctx.enter_context(tc.tile_pool(name="x", bufs=1))
        bigpool = ctx.enter_context(tc.tile_pool(name="big", bufs=1))
        expool = ctx.enter_context(tc.tile_pool(name="e", bufs=4))
        rpool = ctx.enter_context(tc.tile_pool(name="r", bufs=2))
        opool = ctx.enter_context(tc.tile_pool(name="o", bufs=3))
        dpool = ctx.enter_context(tc.tile_pool(name="d", bufs=2, space="DRAM"))
        # PSUM: spsum 2x[128,1024]=4 banks; pv 4x[128,512]=4 banks
        spsum = ctx.enter_context(tc.tile_pool(name="sp", bufs=2, space="PSUM"))
        pvpsum = ctx.enter_context(tc.tile_pool(name="pv", bufs=4, space="PSUM"))

        # --- weights / biases / x loads; first S-chunk's x + weights first ---
        xt = [xpool.tile([128, S], F16, tag=f"xt{et}", name=f"xt{et}")
              for et in range(ET)]
        w_sb = {}
        for name, src in (("wq", wq), ("wk", wk), ("wv", wv)):
            w_sb[name] = wpool.tile([128, ET, HC], F16, tag=name, name=name)
        nc.sync.dma_start(out=w_sb["wq"],
                          in_=wq.rearrange("p (a c) -> p a c", c=HC))
        for et in range(ET):
            nc.sync.dma_start(
                out=xt[et][:, 0:512], in_=xT[et * 128:(et + 1) * 128, 0:512])
        for name, src in (("wk", wk), ("wv", wv)):
            nc.sync.dma_start(out=w_sb[name],
                              in_=src.rearrange("p (a c) -> p a c", c=HC))
        b_sb = {}
        for name, src in (("bq", bq), ("bk", bk), ("bv", bv)):
            t = wpool.tile([HC, 1], F32, tag=name, name=name)
            nc.sync.dma_start(out=t, in_=src[:, :])
            b_sb[name] = t
        for sc in range(1, NSC):
            for et in range(ET):
                nc.sync.dma_start(
                    out=xt[et][:, sc * 512:(sc + 1) * 512],
                    in_=xT[et * 128:(et + 1) * 128, sc * 512:(sc + 1) * 512])
        wo_sb = wpool.tile([HC, E], F16, tag="wo")
        nc.sync.dma_start(out=wo_sb, in_=wo[:, :])
        QKT = bigpool.tile([128, 2 * S], F16, tag="qkt")   # [:, 0:S]=Q, [S:2S]=K
        VT = bigpool.tile([128, S], F16, tag="vt")
        ATT = bigpool.tile([128, S], F16, tag="att")
        # padded to 128 so DMA-transpose outputs land 128-aligned
        V65 = [bigpool.tile([128, NKT, 128], F16, tag=f"v65_{h}", name=f"v65_{h}")
               for h in range(2)]
        for h in range(2):
            nc.vector.memset(V65[h][:, :, 64:65], 1.0)
        ones33 = wpool.tile([33, 64], F16, tag="ones33")
        nc.vector.memset(ones33[0:1, :], 1.0)
        nc.vector.memset(ones33[32:33, :], 1.0)
        # persistent l-row staging (fp16, carrying l/64 so 1/x stays normal);
        # middle rows memset so the full-tile reciprocal reads no garbage
        lr_t = wpool.tile([33, 512], F32, tag="lr")
        nc.vector.memset(lr_t[:, :], 1.0)
        rcp32_t = wpool.tile([33, 512], F32, tag="rcp32")
        rcp_t = wpool.tile([33, 512], F16, tag="rcp")

        # --- stage A: projections, 512-wide S chunks ---
        for sc in range(NSC):
            ssl = slice(sc * 512, (sc + 1) * 512)
            big1 = spsum.tile([128, 1024], F32, tag="sb")
            big2 = pvpsum.tile([128, 512], F32, tag="pv")
            for et in range(ET):
                xsl = xt[et][:, ssl]
                first, last = et == 0, et == ET - 1
                nc.tensor.matmul(big1[:, 0:512], lhsT=w_sb["wq"][:, et, :],
                                 rhs=xsl, start=first, stop=last)
                nc.tensor.matmul(big1[:, 512:1024], lhsT=w_sb["wk"][:, et, :],
                                 rhs=xsl, start=first, stop=last)
                nc.tensor.matmul(big2, lhsT=w_sb["wv"][:, et, :],
                                 rhs=xsl, start=first, stop=last)
            nc.vector.tensor_scalar_add(QKT[:, ssl], big1[:, 0:512], b_sb["bq"])
            nc.vector.tensor_scalar_add(QKT[:, S + sc * 512:S + (sc + 1) * 512],
                                        big1[:, 512:1024], b_sb["bk"])
            nc.vector.tensor_scalar_add(VT[:, ssl], big2, b_sb["bv"])
            # stage B: V65 via DMA transpose of freshly-written VT columns
            for h in range(2):
                nc.sync.dma_start_transpose(
                    out=V65[h][:, 4 * sc:4 * sc + 4, 0:64],
                    in_=VT[h * 64:(h + 1) * 64, ssl],
                )

        # --- stage C+D: attention per 512-q chunk; out-proj deferred 1 chunk ---
        def outproj_qs(qq, qs):
            po = spsum.tile([128, 1024], F32, tag="sb", name="po")
            at = ATT[:, qq * 512 + qs * 128:qq * 512 + (qs + 1) * 128]
            nc.tensor.matmul(po[:, 0:512], lhsT=at, rhs=wo_sb[:, 0:512],
                             start=True, stop=True)
            nc.tensor.matmul(po[:, 512:1024], lhsT=at, rhs=wo_sb[:, 512:1024],
                             start=True, stop=True)
            osb = opool.tile([128, 1024], F16, tag="osb", name="osb")
            # alternate the PSUM evacuation engine so neither exp stream stalls
            if qs % 2 == 0:
                nc.scalar.copy(osb, po)
            else:
                nc.vector.tensor_copy(osb, po)
            nc.sync.dma_start(
                out=out[qq * 512 + qs * 128:qq * 512 + (qs + 1) * 128, :],
                in_=osb,
            )

        pv_prev = None
        ns_state = {}

        def normalize_step(qq, pvq, step):
            qsl2 = slice(qq * 512, (qq + 1) * 512)
            if step == 0:
                # evacuate pv (scaled 1/64) to SBUF fp16, freeing PSUM banks;
                # both heads stacked so the final multiply is one op
                pvc = rpool.tile([128, 512], F16, tag="pvc", name="pvc")
                nc.scalar.mul(pvc[0:64, :], pvq[0][0:64, :], 1.0 / 64.0)
                with nc.allow_low_precision(reason="fp16 attn out"):
                    nc.vector.tensor_scalar_mul(pvc[64:128, :],
                                                pvq[1][0:64, :], 1.0 / 64.0)
                ns_state["pvc"] = pvc
            elif step == 1:
                for h in range(2):
                    nc.scalar.mul(lr_t[32 * h:32 * h + 1, :], pvq[h][64:65, :],
                                  1.0 / 64.0)
            elif step == 2:
                nc.vector.reciprocal_approx_fast(rcp32_t, lr_t)
                nc.scalar.copy(rcp_t, rcp32_t)
            elif step == 3:
                # PE broadcast of the 64/l rows across partitions
                bcps = pvpsum.tile([128, 512], F32, tag="pv", name="bcps")
                for h in range(2):
                    nc.tensor.matmul(bcps[h * 64:(h + 1) * 64, :],
                                     lhsT=ones33[32 * h:32 * h + 1, :],
                                     rhs=rcp_t[32 * h:32 * h + 1, :],
                                     start=True, stop=True)
                bc = rpool.tile([128, 512], F16, tag="bc", name="bc")
                nc.scalar.copy(bc, bcps)
                ns_state["bc"] = bc
            else:
                nc.vector.tensor_mul(ATT[:, qsl2], ns_state["pvc"][:, :],
                                     ns_state["bc"][:, :])

        for qc in range(NSC):
            qsl = slice(qc * 512, (qc + 1) * 512)
            pv = [pvpsum.tile([128, 512], F32, tag="pv", name=f"pv{h}")
                  for h in range(2)]
            exq = {}

            def pv_mm(kt):
                ex = exq.pop(kt)
                for h in range(2):
                    nc.tensor.matmul(
                        pv[h][0:65, :], lhsT=V65[h][:, kt, 0:65],
                        rhs=ex[:, h * 512:(h + 1) * 512],
                        start=(kt == 0), stop=(kt == NKT - 1),
                    )

            for kt in range(NKT):
                sb = spsum.tile([128, 1024], F32, tag="sb")
                for h in range(2):
                    hsl = slice(h * 64, (h + 1) * 64)
                    nc.tensor.matmul(
                        sb[:, h * 512:(h + 1) * 512],
                        lhsT=QKT[hsl, S + kt * 128:S + (kt + 1) * 128],
                        rhs=QKT[hsl, qsl],
                        start=True, stop=True,
                    )
                ex = expool.tile([128, 1024], F16, tag="ex")
                if kt % 2 == 0:
                    nc.scalar.activation(
                        ex, sb, mybir.ActivationFunctionType.Exp, scale=0.125)
                else:
                    nc.vector.tensor_scalar(
                        ex[:, :].bitcast(I16), sb[:, :], A16, B16,
                        mybir.AluOpType.mult, mybir.AluOpType.add)
                exq[kt] = ex
                # PV runs 2-3 slots behind (paired bursts) so exp latency
                # stays hidden and LDWEIGHTS chains stay dense
                if kt >= 3 and kt % 2 == 1:
                    pv_mm(kt - 3)
                    pv_mm(kt - 2)
                # previous chunk's normalize + out-projection, spread across
                # this chunk's slots so they never precede our score matmuls
                # in any engine stream
                if pv_prev is not None:
                    if kt in (2, 3, 5, 9, 11):
                        normalize_step(qc - 1, pv_prev,
                                       {2: 0, 3: 1, 5: 2, 9: 3, 11: 4}[kt])
                    elif kt in (16, 20, 24, 28):
                        outproj_qs(qc - 1, (kt - 16) // 4)
            pv_mm(NKT - 2)
            pv_mm(NKT - 1)
            pv_prev = pv
        # tail: last chunk's normalize + out-projection
        for step in range(5):
            normalize_step(NSC - 1, pv_prev, step)
        for qs in range(NQS):
            outproj_qs(NSC - 1, qs)
    nc.finalize()
    return nc


def _get_nc(S=SEQ):
    key = S
    if key not in _NC_CACHE:
        _NC_CACHE[key] = _build_nc(S=S)
    return _NC_CACHE[key]


def _rearr_w(w):
    # [E, HC] -> [128, ET*HC]: w_r[p, a*HC + c] = w[a*128 + p, c]
    et = EMBED // 128
    return np.ascontiguousarray(
        w.reshape(et, 128, HC).transpose(1, 0, 2).reshape(128, et * HC))


def _make_in_maps(x, Wq, bq, Wk, bk, Wv, bv, Wo):
    xT = np.ascontiguousarray(np.asarray(x, np.float32)[0].T.astype(np.float16))
    Wq, Wk, Wv, Wo = (np.asarray(a, np.float32).astype(np.float16)
                      for a in (Wq, Wk, Wv, Wo))
    bq, bk, bv = (np.asarray(a, np.float32) for a in (bq, bk, bv))
    in_maps = []
    for c in range(N_CORES):
        sl = slice(c * HC, (c + 1) * HC)
        in_maps.append({
            "xT": xT,
            "wq": _rearr_w(Wq[:, sl]),
            "wk": _rearr_w(Wk[:, sl]),
            "wv": _rearr_w(Wv[:, sl]),
            "bq": np.ascontiguousarray(bq[sl]).reshape(HC, 1),
            "bk": np.ascontiguousarray(bk[sl]).reshape(HC, 1),
            "bv": np.ascontiguousarray(bv[sl]).reshape(HC, 1),
            "wo": np.ascontiguousarray(Wo[sl, :]),
        })
    return in_maps


def run(inputs, trace=False):
    """Run the kernel; returns (out [1,S,E] float32, BassKernelResults)."""
    from concourse.bass_utils import run_bass_kernel_spmd

    nc = _get_nc()
    in_maps = _make_in_maps(
        inputs["x"], inputs["Wq"], inputs["bq"], inputs["Wk"], inputs["bk"],
        inputs["Wv"], inputs["bv"], inputs["Wo"],
    )
    res = run_bass_kernel_spmd(
        nc, in_maps, core_ids=list(range(N_CORES)), trace=trace
    )
    acc = np.zeros((SEQ, EMBED), np.float64)
    for c in range(N_CORES):
        acc += res.results[c]["out"].astype(np.float64)
    acc += np.asarray(inputs["bo"], np.float64)
    return acc.astype(np.float32).reshape(1, SEQ, EMBED), res


def kernel(x, Wq, bq, Wk, bk, Wv, bv, Wo, bo):
    out, _ = run(dict(x=x, Wq=Wq, bq=bq, Wk=Wk, bk=bk, Wv=Wv, bv=bv, Wo=Wo, bo=bo))
    return out


# revision 41
# speedup vs baseline: 1.6713x; 1.0230x over previous
"""TRN2 Bass/Tile kernel: 16-head MHA, B=1 S=4096 E=1024, head-sharded over 8 cores.

Sharding: tensor-parallel over heads. Core c owns heads {2c, 2c+1}: columns
[128c, 128(c+1)) of Wq/Wk/Wv (+bias slices) and rows [128c, 128(c+1)) of Wo.
Each core computes attention for its 2 heads and a partial out-projection
[S, E] in fp16; the host sums the 8 partials and adds bo.

v2 pipeline (all fp16 matmuls, fp32 PSUM):
  A) QT/KT [128ch, S] = W^T x (row 0:64 = head0 ch, 64:128 = head1 ch), VT same.
     Bias folded into the PSUM->SBUF evacuation (tensor_scalar_add).
  B) V65[h] [128k, kt, 65] = [V_h | ones] via DMA transpose of VT.
  C) scores^T [k, q]: TWO row-tiled concurrent matmuls (c=64 per head,
     tile_position (0,0)/(64,0)) into one [128, 1024] PSUM pair.
     exp: even kt on ACT (exact, scale=1/8), odd kt on DVE via fp16
     Schraudolph (one tensor_scalar mult+add -> int16 bitcast).
     PV: [65, 512] += V65^T ex accumulated over kt; row 64 = softmax denom.
  D) recip(l) -> DRAM bounce partition-broadcast -> scale -> ATT [128ch, S]
     -> out-proj ATT^T Wo per 128-q tile -> fp16 out.
"""

import sys

for _p in ("/opt/trn_rl_repo", "/opt/pypackages"):
    if _p not in sys.path:
        sys.path.append(_p)

import numpy as np

EMBED = 1024
N_CORES = 8
HC = EMBED // N_CORES  # 128 channels = 2 heads per core
DH = 64                # head dim
SEQ = 4096

# fp16 Schraudolph: exp(s/8) ~= bitcast_f16(int16(A16*s + B16))
A16 = 1024 * 1.4426950408889634 / 8.0
B16 = 15360.0 - 44.25

_NC_CACHE = {}


def _build_nc(S=SEQ, E=EMBED):
    from contextlib import ExitStack

    import concourse.bass as bass
    import concourse.mybir as mybir
    import concourse.tile as tile
    from concourse import bacc

    F32 = mybir.dt.float32
    F16 = mybir.dt.float16
    I16 = mybir.dt.int16

    ET = E // 128      # 8 contraction tiles for projections
    NSC = S // 512     # 8 S-chunks
    NKT = S // 128     # 32 key tiles
    NQS = 512 // 128   # 128-q subtiles per chunk

    nc = bacc.Bacc()
    xT = nc.declare_dram_parameter("xT", [E, S], F16, isOutput=False)
    # host pre-rearranged to [128, ET*HC] so each partition is one 2KB line
    wq = nc.declare_dram_parameter("wq", [128, ET * HC], F16, isOutput=False)
    wk = nc.declare_dram_parameter("wk", [128, ET * HC], F16, isOutput=False)
    wv = nc.declare_dram_parameter("wv", [128, ET * HC], F16, isOutput=False)
    bq = nc.declare_dram_parameter("bq", [HC, 1], F32, isOutput=False)
    bk = nc.declare_dram_parameter("bk", [HC, 1], F32, isOutput=False)
    bv = nc.declare_dram_parameter("bv", [HC, 1], F32, isOutput=False)
    wo = nc.declare_dram_parameter("wo", [HC, E], F16, isOutput=False)
    out = nc.declare_dram_parameter("out", [S, E], F16, isOutput=True)

    with tile.TileContext(nc) as tc, ExitStack() as ctx:
        wpool = ctx.enter_context(tc.tile_pool(name="w", bufs=1))
        xpool = ctx.enter_context(tc.tile_pool(name="x", bufs=1))
        bigpool = ctx.enter_context(tc.tile_pool(name="big", bufs=1))
        expool = ctx.enter_context(tc.tile_pool(name="e", bufs=4))
        rpool = ctx.enter_context(tc.tile_pool(name="r", bufs=2))
        opool = ctx.enter_context(tc.tile_pool(name="o", bufs=3))
        dpool = ctx.enter_context(tc.tile_pool(name="d", bufs=2, space="DRAM"))
        # PSUM: spsum 2x[128,1024]=4 banks; pv 4x[128,512]=4 banks
        spsum = ctx.enter_context(tc.tile_pool(name="sp", bufs=2, space="PSUM"))
        pvpsum = ctx.enter_context(tc.tile_pool(name="pv", bufs=4, space="PSUM"))

        # --- weights / biases / x loads; first S-chunk's x + weights first ---
        xt = [xpool.tile([128, S], F16, tag=f"xt{et}", name=f"xt{et}")
              for et in range(ET)]
        w_sb = {}
        for name, src in (("wq", wq), ("wk", wk), ("wv", wv)):
            w_sb[name] = wpool.tile([128, ET, HC], F16, tag=name, name=name)
        for name, src in (("wq", wq), ("wk", wk), ("wv", wv)):
            nc.sync.dma_start(out=w_sb[name],
                              in_=src.rearrange("p (a c) -> p a c", c=HC))
        for et in range(ET):
            nc.sync.dma_start(
                out=xt[et][:, 0:512], in_=xT[et * 128:(et + 1) * 128, 0:512])
        b_sb = {}
        for name, src in (("bq", bq), ("bk", bk), ("bv", bv)):
            t = wpool.tile([HC, 1], F32, tag=name, name=name)
            nc.sync.dma_start(out=t, in_=src[:, :])
            b_sb[name] = t
        for sc in range(1, NSC):
            for et in range(ET):
                nc.sync.dma_start(
                    out=xt[et][:, sc * 512:(sc + 1) * 512],
                    in_=xT[et * 128:(et + 1) * 128, sc * 512:(sc + 1) * 512])
        wo_sb = wpool.tile([HC, E], F16, tag="wo")
        nc.sync.dma_start(out=wo_sb, in_=wo[:, :])
        QKT = bigpool.tile([128, 2 * S], F16, tag="qkt")   # [:, 0:S]=Q, [S:2S]=K
        VT = bigpool.tile([128, S], F16, tag="vt")
        ATT = bigpool.tile([128, S], F16, tag="att")
        # padded to 128 so DMA-transpose outputs land 128-aligned
        V65 = [bigpool.tile([128, NKT, 128], F16, tag=f"v65_{h}", name=f"v65_{h}")
               for h in range(2)]
        for h in range(2):
            nc.vector.memset(V65[h][:, :, 64:65], 1.0)
        ones33 = wpool.tile([33, 64], F16, tag="ones33")
        nc.vector.memset(ones33[0:1, :], 1.0)
        nc.vector.memset(ones33[32:33, :], 1.0)
        # persistent l-row staging (fp16, carrying l/64 so 1/x stays normal);
        # middle rows memset so the full-tile reciprocal reads no garbage
        lr_t = wpool.tile([33, 512], F32, tag="lr")
        nc.vector.memset(lr_t[:, :], 1.0)
        rcp32_t = wpool.tile([33, 512], F32, tag="rcp32")
        rcp_t = wpool.tile([33, 512], F16, tag="rcp")

        # --- stage A: projections, 512-wide S chunks ---
        for sc in range(NSC):
            ssl = slice(sc * 512, (sc + 1) * 512)
            big1 = spsum.tile([128, 1024], F32, tag="sb")
            big2 = pvpsum.tile([128, 512], F32, tag="pv")
            for et in range(ET):
                xsl = xt[et][:, ssl]
                first, last = et == 0, et == ET - 1
                nc.tensor.matmul(big1[:, 0:512], lhsT=w_sb["wq"][:, et, :],
                                 rhs=xsl, start=first, stop=last)
                nc.tensor.matmul(big1[:, 512:1024], lhsT=w_sb["wk"][:, et, :],
                                 rhs=xsl, start=first, stop=last)
                nc.tensor.matmul(big2, lhsT=w_sb["wv"][:, et, :],
                                 rhs=xsl, start=first, stop=last)
            nc.vector.tensor_scalar_add(QKT[:, ssl], big1[:, 0:512], b_sb["bq"])
            nc.vector.tensor_scalar_add(QKT[:, S + sc * 512:S + (sc + 1) * 512],
                                        big1[:, 512:1024], b_sb["bk"])
            nc.vector.tensor_scalar_add(VT[:, ssl], big2, b_sb["bv"])
            # stage B: V65 via DMA transpose of freshly-written VT columns
            for h in range(2):
                nc.sync.dma_start_transpose(
                    out=V65[h][:, 4 * sc:4 * sc + 4, 0:64],
                    in_=VT[h * 64:(h + 1) * 64, ssl],
                )

        # --- stage C+D: attention per 512-q chunk; out-proj deferred 1 chunk ---
        def outproj_qs(qq, qs):
            po = spsum.tile([128, 1024], F32, tag="sb", name="po")
            at = ATT[:, qq * 512 + qs * 128:qq * 512 + (qs + 1) * 128]
            nc.tensor.matmul(po[:, 0:512], lhsT=at, rhs=wo_sb[:, 0:512],
                             start=True, stop=True)
            nc.tensor.matmul(po[:, 512:1024], lhsT=at, rhs=wo_sb[:, 512:1024],
                             start=True, stop=True)
            osb = opool.tile([128, 1024], F16, tag="osb", name="osb")
            nc.scalar.copy(osb, po)
            nc.sync.dma_start(
                out=out[qq * 512 + qs * 128:qq * 512 + (qs + 1) * 128, :],
                in_=osb,
            )

        pv_prev = None
        ns_state = {}

        def normalize_step(qq, pvq, step):
            qsl2 = slice(qq * 512, (qq + 1) * 512)
            if step == 0:
                # evacuate pv (scaled 1/64) to SBUF fp16, freeing PSUM banks;
                # both heads stacked so the final multiply is one op
                pvc = rpool.tile([128, 512], F16, tag="pvc", name="pvc")
                nc.scalar.mul(pvc[0:64, :], pvq[0][0:64, :], 1.0 / 64.0)
                with nc.allow_low_precision(reason="fp16 attn out"):
                    nc.vector.tensor_scalar_mul(pvc[64:128, :],
                                                pvq[1][0:64, :], 1.0 / 64.0)
                ns_state["pvc"] = pvc
            elif step == 1:
                for h in range(2):
                    nc.scalar.mul(lr_t[32 * h:32 * h + 1, :], pvq[h][64:65, :],
                                  1.0 / 64.0)
            elif step == 2:
                nc.vector.reciprocal_approx_fast(rcp32_t, lr_t)
                nc.scalar.copy(rcp_t, rcp32_t)
            elif step == 3:
                # PE broadcast of the 64/l rows across partitions
                bcps = pvpsum.tile([128, 512], F32, tag="pv", name="bcps")
                for h in range(2):
                    nc.tensor.matmul(bcps[h * 64:(h + 1) * 64, :],
                                     lhsT=ones33[32 * h:32 * h + 1, :],
                                     rhs=rcp_t[32 * h:32 * h + 1, :],
                                     start=True, stop=True)
                bc = rpool.tile([128, 512], F16, tag="bc", name="bc")
                nc.scalar.copy(bc, bcps)
                ns_state["bc"] = bc
            else:
                nc.vector.tensor_mul(ATT[:, qsl2], ns_state["pvc"][:, :],
                                     ns_state["bc"][:, :])

        for qc in range(NSC):
            qsl = slice(qc * 512, (qc + 1) * 512)
            pv = [pvpsum.tile([128, 512], F32, tag="pv", name=f"pv{h}")
                  for h in range(2)]
            exq = {}

            def pv_mm(kt):
                ex = exq.pop(kt)
                for h in range(2):
                    nc.tensor.matmul(
                        pv[h][0:65, :], lhsT=V65[h][:, kt, 0:65],
                        rhs=ex[:, h * 512:(h + 1) * 512],
                        start=(kt == 0), stop=(kt == NKT - 1),
                    )

            for kt in range(NKT):
                sb = spsum.tile([128, 1024], F32, tag="sb")
                for h in range(2):
                    hsl = slice(h * 64, (h + 1) * 64)
                    nc.tensor.matmul(
                        sb[:, h * 512:(h + 1) * 512],
                        lhsT=QKT[hsl, S + kt * 128:S + (kt + 1) * 128],
                        rhs=QKT[hsl, qsl],
                        start=True, stop=True,
                    )
                ex = expool.tile([128, 1024], F16, tag="ex")
                if kt % 2 == 0:
                    nc.scalar.activation(
                        ex, sb, mybir.ActivationFunctionType.Exp, scale=0.125)
                else:
                    nc.vector.tensor_scalar(
                        ex[:, :].bitcast(I16), sb[:, :], A16, B16,
                        mybir.AluOpType.mult, mybir.AluOpType.add)
                exq[kt] = ex
                # PV runs 2-3 slots behind (paired bursts) so exp latency
                # stays hidden and LDWEIGHTS chains stay dense
                if kt >= 3 and kt % 2 == 1:
                    pv_mm(kt - 3)
                    pv_mm(kt - 2)
                # previous chunk's normalize + out-projection, spread across
                # this chunk's slots so they never precede our score matmuls
                # in any engine stream
                if pv_prev is not None:
                    if kt in (2, 3, 5, 9, 11):
                        normalize_step(qc - 1, pv_prev,
                                       {2: 0, 3: 1, 5: 2, 9: 3, 11: 4}[kt])
                    elif kt in (16, 20, 24, 28):
                        outproj_qs(qc - 1, (kt - 16) // 4)
            pv_mm(NKT - 2)
            pv_mm(NKT - 1)
            pv_prev = pv
        # tail: last chunk's normalize + out-projection
        for step in range(5):
            normalize_step(NSC - 1, pv_prev, step)
        for qs in range(NQS):
            outproj_qs(NSC - 1, qs)
    nc.finalize()
    return nc


def _get_nc(S=SEQ):
    key = S
    if key not in _NC_CACHE:
        _NC_CACHE[key] = _build_nc(S=S)
    return _NC_CACHE[key]


def _rearr_w(w):
    # [E, HC] -> [128, ET*HC]: w_r[p, a*HC + c] = w[a*128 + p, c]
    et = EMBED // 128
    return np.ascontiguousarray(
        w.reshape(et, 128, HC).transpose(1, 0, 2).reshape(128, et * HC))


def _make_in_maps(x, Wq, bq, Wk, bk, Wv, bv, Wo):
    xT = np.ascontiguousarray(np.asarray(x, np.float32)[0].T.astype(np.float16))
    Wq, Wk, Wv, Wo = (np.asarray(a, np.float32).astype(np.float16)
                      for a in (Wq, Wk, Wv, Wo))
    bq, bk, bv = (np.asarray(a, np.float32) for a in (bq, bk, bv))
    in_maps = []
    for c in range(N_CORES):
        sl = slice(c * HC, (c + 1) * HC)
        in_maps.append({
            "xT": xT,
            "wq": _rearr_w(Wq[:, sl]),
            "wk": _rearr_w(Wk[:, sl]),
            "wv": _rearr_w(Wv[:, sl]),
            "bq": np.ascontiguousarray(bq[sl]).reshape(HC, 1),
            "bk": np.ascontiguousarray(bk[sl]).reshape(HC, 1),
            "bv": np.ascontiguousarray(bv[sl]).reshape(HC, 1),
            "wo": np.ascontiguousarray(Wo[sl, :]),
        })
    return in_maps


def run(inputs, trace=False):
    """Run the kernel; returns (out [1,S,E] float32, BassKernelResults)."""
    from concourse.bass_utils import run_bass_kernel_spmd

    nc = _get_nc()
    in_maps = _make_in_maps(
        inputs["x"], inputs["Wq"], inputs["bq"], inputs["Wk"], inputs["bk"],
        inputs["Wv"], inputs["bv"], inputs["Wo"],
    )
    res = run_bass_kernel_spmd(
        nc, in_maps, core_ids=list(range(N_CORES)), trace=trace
    )
    acc = np.zeros((SEQ, EMBED), np.float64)
    for c in range(N_CORES):
        acc += res.results[c]["out"].astype(np.float64)
    acc += np.asarray(inputs["bo"], np.float64)
    return acc.astype(np.float32).reshape(1, SEQ, EMBED), res


def kernel(x, Wq, bq, Wk, bk, Wv, bv, Wo, bo):
    out, _ = run(dict(x=x, Wq=Wq, bq=bq, Wk=Wk, bk=bk, Wv=Wv, bv=bv, Wo=Wo, bo=bo))
    return out
